# revision 17
# baseline (speedup 1.0000x reference)
"""DFlash Qwen3 cross-attention on 8 TRN2 NeuronCores (v3).

Sharding: tensor-parallel over heads. Core c owns KV head c (KVH=8) and the
4 query heads 4c..4c+3 of its GQA group. Each core computes its heads'
QKV projections, per-head RMSNorm + RoPE, causal attention; per q-tile j the
normalized attention outputs (transposed, [4*D, 512]) are AllGathered so
every core holds attn^T for all 32 heads; each core computes a 512-column
slice of o_proj and the host concatenates the 8 slices.

v3 structure (vs v2, driven by the NTFF profile of v2):
- phase order: context proj (16 groups) -> query proj (groups 1..7 then 0)
  -> attention j=3,2,1,0 -> o_proj j' trailing one attention block. The last
  attention block is the cheapest (j=0), and o_proj(1) hides AG(0) latency.
- bubble-free attention quads: both score-pair matmuls emitted before the
  exps, AV matmuls after, so the PE never waits a full exp latency mid-quad.
- stream finalize (denominator matmuls, reciprocal, normalize, AllGather
  payload write) deferred into the NEXT stream after its first quad: kills
  the ~2.8us PE stall + HAM re-throttle at every stream boundary.
- per-j single AllGather with a [128, 4(qc), 4(h), 128] payload so o_proj
  reads back [128, 8c, 4h, 128] tiles as 1KB-contiguous segments, prefetched
  double-buffered on the idle sync DMA queue.
- RMSNorm rsqrt fully on DVE (quake bit-trick + 1 Newton step): the ACT
  engine runs exp only -- no ACT table swaps (v2 paid 27 x 1.3us mid-attention).
- wkv weight DMA split in 4 + non-critical loads (wq/wo/msk) deferred past
  the startup barrier window so the first matmul issues ~40us earlier.
"""

from contextlib import ExitStack

import numpy as np
from ml_dtypes import bfloat16

import concourse.bass as bass
import concourse.bass_isa as bass_isa
import concourse.mybir as mybir
import concourse.tile as tile
from concourse import bacc
from concourse.bass_utils import run_bass_kernel_spmd
from concourse.masks import make_identity

H = 32
KVH = 8
D = 128
HID = 4096
CTX = 4096
QL = 2048
KV = CTX + QL  # 6144
NCORES = 8
HPC = H // NCORES  # 4 q heads per core
THETA = 1000000.0
EPS = 1e-6
SCALE = float(D) ** -0.5

NHD = HID // 128  # 32 contraction chunks
NKV = KV // 128  # 48 kv chunks
NQC = QL // 128  # 16 q row chunks
NQJ = QL // 512  # 4 q column tiles for attention
NCTX = CTX // 128  # 32 context kv chunks
MASKVAL = -1e6
MAGIC = 0x5F3759DF  # fast inverse sqrt seed

F32 = mybir.dt.float32
BF16 = mybir.dt.bfloat16
I32 = mybir.dt.int32
ALU = mybir.AluOpType

_STATE = {}


def _build():
    nc = bacc.Bacc()

    ckT3 = nc.declare_dram_parameter("ckT3", [128, NHD, KV], BF16, isOutput=False)
    wq3 = nc.declare_dram_parameter("wq3", [128, NHD, HPC * D], BF16, isOutput=False)
    wkv3 = nc.declare_dram_parameter("wkv3", [128, NHD, 2 * D], BF16, isOutput=False)
    wo3 = nc.declare_dram_parameter("wo3", [128, NHD, HPC * D], BF16, isOutput=False)
    csq = nc.declare_dram_parameter("csq", [128, NQC, 2 * D], BF16, isOutput=False)
    csk = nc.declare_dram_parameter("csk", [128, NKV, 2 * D], BF16, isOutput=False)
    mskd = nc.declare_dram_parameter("msk", [128, 896], BF16, isOutput=False)
    out_ext = nc.declare_dram_parameter("out", [QL, HPC * D], F32, isOutput=True)

    warm_in = nc.dram_tensor("warm_in", [128, 8], BF16)
    warm_out = nc.dram_tensor("warm_out", [NCORES * 128, 8], BF16, addr_space="Shared")
    # AllGather payload per j: [128 d-part, 4 qc, 4 h, 128 q] so the o_proj
    # readback for one qc is 1KB-contiguous per core block. j=0 (the LAST
    # attention block) gathers per head instead so only a 1MB collective and
    # the h=3 quarter of o_proj(0) remain on the critical tail.
    ag_ins = [nc.dram_tensor(f"ag_in{j}", [128, 4, HPC, 128], BF16)
              for j in range(1, NQJ)]
    ag_outs = [nc.dram_tensor(f"ag_out{j}", [NCORES * 128, 4, HPC, 128], BF16,
                              addr_space="Shared") for j in range(1, NQJ)]
    ag0_ins = [nc.dram_tensor(f"ag0_in{h}", [128, 4, 128], BF16)
               for h in range(HPC)]
    ag0_outs = [nc.dram_tensor(f"ag0_out{h}", [NCORES * 128, 4, 128], BF16,
                               addr_space="Shared") for h in range(HPC)]

    with tile.TileContext(nc) as tc, ExitStack() as ctx:
        singles = ctx.enter_context(tc.tile_pool(name="singles", bufs=1))
        ckv_pool = ctx.enter_context(tc.tile_pool(name="ckv", bufs=3))
        evac_pool = ctx.enter_context(tc.tile_pool(name="evac", bufs=3))
        work_pool = ctx.enter_context(tc.tile_pool(name="work", bufs=2))
        pt_pool = ctx.enter_context(tc.tile_pool(name="ptp", bufs=3))
        saccs_pool = ctx.enter_context(tc.tile_pool(name="sac", bufs=2))
        stg_pool = ctx.enter_context(tc.tile_pool(name="stg", bufs=2))
        at_pool = ctx.enter_context(tc.tile_pool(name="atp", bufs=2))
        # PSUM: 4 x 1-bank accumulators + 2 x 2-bank score tiles = 8 banks
        p1 = ctx.enter_context(tc.tile_pool(name="p1", bufs=4, space="PSUM"))
        p2 = ctx.enter_context(tc.tile_pool(name="p2", bufs=2, space="PSUM"))

        # ---- critical-path DMAs first: wkv (split x4) + first ck chunks ----
        wkv_sb = singles.tile([128, NHD, 2 * D], BF16)
        for q4 in range(4):
            nc.scalar.dma_start(out=wkv_sb[:, q4 * 8:(q4 + 1) * 8, :],
                                in_=wkv3[:, q4 * 8:(q4 + 1) * 8, :])

        def load_ckv(c0):
            """Stream ckT3[:, :, c0:c0+256] in two k-halves (sync queue, so
            the warmup collective trigger on gpsimd can't block them)."""
            ca = ckv_pool.tile([128, NHD // 2, 256], BF16, tag="ckv")
            nc.sync.dma_start(out=ca, in_=ckT3[:, 0:NHD // 2, c0:c0 + 256])
            cb = ckv_pool.tile([128, NHD // 2, 256], BF16, tag="ckv")
            nc.sync.dma_start(out=cb, in_=ckT3[:, NHD // 2:NHD, c0:c0 + 256])

            def sl(k, c):
                t = ca if k < NHD // 2 else cb
                return t[:, k % (NHD // 2), c * 128:(c + 1) * 128]
            return sl

        # ---- warmup collective: pay the first-collective handshake now ----
        wtile = singles.tile([128, 8], BF16)
        nc.vector.memset(wtile, 0.0)
        nc.gpsimd.dma_start(out=warm_in[:, :], in_=wtile)
        nc.gpsimd.collective_compute(
            "AllGather", ALU.bypass, ins=[warm_in[:, :]], outs=[warm_out[:, :]],
            replica_groups=[list(range(NCORES))])

        # ---- constants ----
        identb = singles.tile([128, 128], BF16)
        make_identity(nc, identb)
        ones_bf = singles.tile([128, 128], BF16)
        nc.vector.memset(ones_bf, 1.0)
        zbias = singles.tile([128, 1], F32)
        nc.vector.memset(zbias, 0.0)
        magict = singles.tile([128, 4], I32)
        nc.vector.memset(magict, MAGIC)

        wq_sb = singles.tile([128, NHD, HPC * D], BF16)  # loaded mid-cg
        # compact causal mask: msk_sb[p, 384-128i + q] = 0 if 128i+p <= q
        # else MASKVAL, so chunk i uses the slice [384-128i : 896-128i].
        msk_sb = singles.tile([128, 896], BF16)  # loaded after cg0
        wo_sb = singles.tile([128, NHD, HPC * D], BF16)  # loaded mid-qg

        qT_sb = singles.tile([128, HPC, QL], BF16)  # Q^T per head: [d, h, q]
        kT_sb = singles.tile([128, KV], BF16)  # K^T: [d, kv]
        v_sb = singles.tile([128, NKV, D], BF16)  # V: [kv%128, r, d]

        def rsqrt_sum(rr, ssum, n, tg):
            """rr = 1/sqrt(ssum/D + eps), all on DVE (no ACT table swap)."""
            x = work_pool.tile([128, n], F32, tag=f"rx{tg}")
            nc.vector.tensor_scalar(out=x, in0=ssum, scalar1=1.0 / D,
                                    scalar2=EPS, op0=ALU.mult, op1=ALU.add)
            yi = work_pool.tile([128, n], I32, tag=f"ry{tg}")
            nc.vector.tensor_scalar(out=yi, in0=x[:, :].bitcast(I32), scalar1=1,
                                    scalar2=None, op0=ALU.logical_shift_right)
            nc.vector.tensor_tensor(out=yi, in0=magict[:, 0:n], in1=yi,
                                    op=ALU.subtract)
            y = yi[:, :].bitcast(F32)
            t1 = work_pool.tile([128, n], F32, tag=f"rt{tg}")
            nc.vector.tensor_tensor(out=t1, in0=y, in1=y, op=ALU.mult)
            nc.vector.tensor_tensor(out=t1, in0=t1, in1=x, op=ALU.mult)
            nc.vector.tensor_scalar(out=t1, in0=t1, scalar1=-0.5, scalar2=1.5,
                                    op0=ALU.mult, op1=ALU.add)
            nc.vector.tensor_tensor(out=rr, in0=t1, in1=y, op=ALU.mult)

        def rope(ro, xn, cs, t1):
            """ro = rotate-half(xn) with cs = [cw1|sw2|cw2|sw1] slices."""
            hf = D // 2
            nc.vector.tensor_mul(ro[:, :, 0:hf], xn[:, :, 0:hf], cs[0])
            nc.vector.tensor_mul(t1, xn[:, :, hf:D], cs[1])
            nc.vector.tensor_sub(ro[:, :, 0:hf], ro[:, :, 0:hf], t1)
            nc.vector.tensor_mul(ro[:, :, hf:D], xn[:, :, hf:D], cs[2])
            nc.vector.tensor_mul(t1, xn[:, :, 0:hf], cs[3])
            nc.vector.tensor_add(ro[:, :, hf:D], ro[:, :, hf:D], t1)

        def qnorm(qe, qc, csqp, ci):
            """qe: [128, 4, 128] bf16 SBUF evac of the q projection.
            norm+rope -> 4 transposes -> qT_sb."""
            sq = work_pool.tile([128, HPC, D], BF16, tag="qsq")
            nc.vector.tensor_mul(sq, qe, qe)
            ssum = work_pool.tile([128, HPC], F32, tag="qssum")
            nc.vector.tensor_reduce(ssum, sq, axis=mybir.AxisListType.X, op=ALU.add)
            rr = work_pool.tile([128, HPC], F32, tag="qrr")
            rsqrt_sum(rr, ssum, HPC, "q")
            qn = work_pool.tile([128, HPC, D], BF16, tag="qn")
            for h in range(HPC):
                nc.vector.tensor_scalar_mul(out=qn[:, h, :], in0=qe[:, h, :],
                                            scalar1=rr[:, h:h + 1])
            hf = D // 2
            ro = work_pool.tile([128, HPC, D], BF16, tag="qro")
            t1 = work_pool.tile([128, HPC, hf], BF16, tag="qt1")
            for h in range(HPC):
                cs = [csqp[:, ci:ci + 1, s * hf:(s + 1) * hf] for s in range(4)]
                rope(ro[:, h:h + 1, :], qn[:, h:h + 1, :], cs, t1[:, 0:1, :])
            for h in range(HPC):
                tp = p1.tile([128, 128], BF16, tag="acc", name=f"tpq{qc}_{h}")
                nc.tensor.transpose(tp, ro[:, h, :], identb)
                nc.scalar.copy(out=qT_sb[:, h, qc * 128:(qc + 1) * 128], in_=tp)

        def knorm_pair(ke, r0, cskp):
            """ke: [128, 2, 2D] bf16 SBUF evac of the K|V projection pair."""
            sq = work_pool.tile([128, 2, D], BF16, tag="ksq")
            nc.vector.tensor_mul(sq, ke[:, :, 0:D], ke[:, :, 0:D])
            ssum = work_pool.tile([128, 2], F32, tag="kssum")
            nc.vector.tensor_reduce(ssum, sq, axis=mybir.AxisListType.X, op=ALU.add)
            rr = work_pool.tile([128, 2], F32, tag="krr")
            rsqrt_sum(rr, ssum, 2, "k")
            kn = work_pool.tile([128, 2, D], BF16, tag="kn")
            for c in range(2):
                nc.vector.tensor_scalar_mul(out=kn[:, c, :], in0=ke[:, c, 0:D],
                                            scalar1=rr[:, c:c + 1])
            hf = D // 2
            cs = [cskp[:, :, s * hf:(s + 1) * hf] for s in range(4)]
            ro = work_pool.tile([128, 2, D], BF16, tag="kro")
            t1 = work_pool.tile([128, 2, hf], BF16, tag="kt1")
            rope(ro, kn, cs, t1)
            for c in range(2):
                tp = p1.tile([128, 128], BF16, tag="acc", name=f"tpk{r0}_{c}")
                nc.tensor.transpose(tp, ro[:, c, :], identb)
                nc.scalar.copy(out=kT_sb[:, (r0 + c) * 128:(r0 + c + 1) * 128], in_=tp)
            nc.vector.tensor_copy(out=v_sb[:, r0:r0 + 2, :], in_=ke[:, :, D:2 * D])

        # Deferred-norm pipeline: each projection group's norm+rope+PE
        # transposes are emitted AFTER the NEXT group's matmuls so the DVE
        # chain hides behind tensor work.
        pending = []

        def flush_pending():
            while pending:
                pending.pop(0)()

        def cg_group(g):
            """Context projection: kv chunks 2g, 2g+1."""
            sl = load_ckv(g * 256)
            cskp = evac_pool.tile([128, 2, 2 * D], BF16, tag="cskp")
            nc.scalar.dma_start(out=cskp, in_=csk[:, 2 * g:2 * g + 2, :])
            pk = p1.tile([128, 2, 2 * D], F32, tag="acc", name=f"cgk{g}")
            for k in range(NHD):
                for c in range(2):
                    nc.tensor.matmul(pk[:, c, :], lhsT=sl(k, c),
                                     rhs=wkv_sb[:, k, :],
                                     start=(k == 0 and c == 0),
                                     stop=(k == NHD - 1))
            ke = evac_pool.tile([128, 2, 2 * D], BF16, tag="ke")
            nc.scalar.copy(out=ke[:], in_=pk)
            flush_pending()
            pending.append(lambda ke=ke, r0=2 * g, cskp=cskp: knorm_pair(ke, r0, cskp))

        def qg_group(g):
            """Query-row projection: kv chunks 32+2g, 33+2g (q chunks 2g,
            2g+1): shared stationary feeds both wkv and wq."""
            sl = load_ckv(CTX + g * 256)
            cskp = evac_pool.tile([128, 2, 2 * D], BF16, tag="cskp")
            nc.scalar.dma_start(out=cskp, in_=csk[:, NCTX + 2 * g:NCTX + 2 * g + 2, :])
            csqp = evac_pool.tile([128, 2, 2 * D], BF16, tag="csqp")
            nc.scalar.dma_start(out=csqp, in_=csq[:, 2 * g:2 * g + 2, :])
            pk = p1.tile([128, 2, 2 * D], F32, tag="acc", name=f"qgk{g}")
            pq = [p1.tile([128, HPC * D], F32, tag="acc", name=f"qgq{g}_{c}")
                  for c in range(2)]
            for k in range(NHD):
                for c in range(2):
                    st = sl(k, c)
                    nc.tensor.matmul(pk[:, c, :], lhsT=st, rhs=wkv_sb[:, k, :],
                                     start=(k == 0 and c == 0),
                                     stop=(k == NHD - 1))
                    nc.tensor.matmul(pq[c], lhsT=st, rhs=wq_sb[:, k, :],
                                     start=(k == 0), stop=(k == NHD - 1))
            ke = evac_pool.tile([128, 2, 2 * D], BF16, tag="ke")
            nc.scalar.copy(out=ke[:], in_=pk)
            qes = []
            for c in range(2):
                qe = evac_pool.tile([128, HPC, D], BF16, tag="qe")
                nc.scalar.copy(out=qe[:], in_=pq[c])
                qes.append(qe)
            flush_pending()

            def norm(ke=ke, qes=qes, csqp=csqp, cskp=cskp, g=g):
                knorm_pair(ke, NCTX + 2 * g, cskp)
                for c in range(2):
                    qnorm(qes[c], 2 * g + c, csqp, c)
            pending.append(norm)

        # ---- attention ----
        # finalize (den matmuls, reciprocal, normalize, AG payload write) of
        # the previous stream is deferred and emitted after the next block of
        # dense PE work has started.
        finalize_pending = []

        def flush_finalize():
            while finalize_pending:
                finalize_pending.pop(0)()

        def make_finalize(j, h, o_acc, saccs):
            def fin():
                den = p1.tile([128, 512], F32, tag="acc", name=f"den{j}_{h}")
                for s in range(4):
                    nc.tensor.matmul(den, lhsT=ones_bf,
                                     rhs=saccs[:, s * 512:(s + 1) * 512],
                                     start=(s == 0), stop=(s == 3))
                pr = work_pool.tile([128, 512], F32, tag="dps")
                nc.vector.reciprocal_approx_fast(out=pr, in_=den)
                s3 = stg_pool.tile([128, 512], BF16, tag="stg")
                nc.vector.tensor_mul(s3, o_acc, pr)
                s3v = s3[:, :].rearrange("p (c q) -> p c q", c=4)
                if j == 0:
                    nc.gpsimd.dma_start(out=ag0_ins[h][:, :, :], in_=s3v)
                    nc.gpsimd.collective_compute(
                        "AllGather", ALU.bypass, ins=[ag0_ins[h][:, :, :]],
                        outs=[ag0_outs[h][:, :, :]],
                        replica_groups=[list(range(NCORES))])
                else:
                    nc.gpsimd.dma_start(out=ag_ins[j - 1][:, :, h, :], in_=s3v)
                    if h == HPC - 1:
                        nc.gpsimd.collective_compute(
                            "AllGather", ALU.bypass,
                            ins=[ag_ins[j - 1][:, :, :, :]],
                            outs=[ag_outs[j - 1][:, :, :, :]],
                            replica_groups=[list(range(NCORES))])
            return fin

        def attn_stream(j, h, split_at=None):
            """One (j,h) attention stream over all its kv chunks.
            split_at: optional chunk index at which to flush the projection
            norm pipeline (used once, to hide the last qg group's norm)."""
            r_list = list(range(NCTX + 4 + 4 * j))
            nq = len(r_list) // 4
            qTj = qT_sb[:, h, j * 512:(j + 1) * 512]
            o_acc = p1.tile([128, 512], F32, tag="acc", name=f"o{j}_{h}")
            saccs = saccs_pool.tile([128, 4 * 512], BF16, tag="sacc")
            for qd in range(nq):
                if split_at is not None and qd * 4 == split_at:
                    flush_pending()
                ptw = pt_pool.tile([128, 4 * 512], BF16, tag="ptw")
                sts = []
                for pi in range(2):
                    rA = r_list[qd * 4 + 2 * pi]
                    rB = r_list[qd * 4 + 2 * pi + 1]
                    st = p2.tile([128, 1024], F32, tag="st")
                    nc.tensor.matmul(st[:, 0:512],
                                     lhsT=kT_sb[:, rA * 128:(rA + 1) * 128],
                                     rhs=qTj, start=True, stop=True)
                    nc.tensor.matmul(st[:, 512:1024],
                                     lhsT=kT_sb[:, rB * 128:(rB + 1) * 128],
                                     rhs=qTj, start=True, stop=True)
                    for idx, r in ((0, rA), (1, rB)):
                        i = r - NCTX - 4 * j
                        if 0 <= i <= 3:
                            nc.vector.tensor_add(
                                st[:, idx * 512:(idx + 1) * 512],
                                st[:, idx * 512:(idx + 1) * 512],
                                msk_sb[:, 384 - 128 * i:896 - 128 * i])
                    sts.append(st)
                for pi in range(2):
                    nc.scalar.activation(out=ptw[:, pi * 1024:(pi + 1) * 1024],
                                         in_=sts[pi][:],
                                         func=mybir.ActivationFunctionType.Exp,
                                         bias=zbias, scale=SCALE)
                for slot in range(4):
                    r = r_list[qd * 4 + slot]
                    nc.tensor.matmul(
                        o_acc, lhsT=v_sb[:, r, :],
                        rhs=ptw[:, slot * 512:(slot + 1) * 512],
                        start=(qd == 0 and slot == 0),
                        stop=(qd == nq - 1 and slot == 3))
                if qd == 0:
                    flush_finalize()
                    nc.gpsimd.tensor_copy(out=saccs, in_=ptw)
                else:
                    nc.gpsimd.tensor_add(saccs, saccs, ptw)
            finalize_pending.append(make_finalize(j, h, o_acc, saccs))

        def oproj_block(jq):
            """o_proj for q tile jq (jq >= 1) from the AllGathered attn^T.
            Tiles load per (qc, 4-core half) as 1KB-contiguous segments."""
            ats = {}

            def at_load(hv):
                qc2, lo = hv // 2, (hv % 2) * 4
                at = at_pool.tile([128, 4, HPC, 128], BF16, tag="at")
                nc.sync.dma_start(
                    out=at,
                    in_=ag_outs[jq - 1][lo * 128:(lo + 4) * 128, qc2, :, :]
                    .rearrange("(c p) h q -> p c h q", p=128))
                ats[hv] = at

            at_load(0)
            at_load(1)
            for qc2 in range(4):
                qc = 4 * jq + qc2
                po = p1.tile([128, HPC * D], F32, tag="acc", name=f"po{qc}")
                for half in range(2):
                    at = ats.pop(qc2 * 2 + half)
                    for c4 in range(4):
                        for h2 in range(HPC):
                            gg = (half * 4 + c4) * HPC + h2
                            nc.tensor.matmul(po, lhsT=at[:, c4, h2, :],
                                             rhs=wo_sb[:, gg, :],
                                             start=(gg == 0), stop=(gg == H - 1))
                    if qc2 == 0 and half == 0:
                        flush_finalize()
                    if qc2 * 2 + half + 2 < 8:
                        at_load(qc2 * 2 + half + 2)
                ot = stg_pool.tile([128, HPC * D], F32, tag="ot")
                nc.scalar.copy(out=ot, in_=po)
                nc.sync.dma_start(out=out_ext[qc * 128:(qc + 1) * 128, :], in_=ot)

        def oproj_block0():
            """o_proj for q tile 0, head-major: the h2<3 contraction chunks
            run while AG(0, h=3) is still in flight, so only the last-quarter
            matmuls trail the final collective."""
            pos = [p1.tile([128, HPC * D], F32, tag="acc", name=f"po0_{qc2}")
                   for qc2 in range(4)]
            ats = {}

            def at0_load(h2, qc2):
                at = at_pool.tile([128, NCORES, 128], BF16, tag="at")
                nc.sync.dma_start(
                    out=at,
                    in_=ag0_outs[h2][:, qc2, :].rearrange("(c p) q -> p c q",
                                                          p=128))
                ats[(h2, qc2)] = at

            at0_load(0, 0)
            at0_load(0, 1)
            order = [(h2, qc2) for h2 in range(HPC) for qc2 in range(4)]
            for oi, (h2, qc2) in enumerate(order):
                at = ats.pop((h2, qc2))
                po = pos[qc2]
                for cc in range(NCORES):
                    nc.tensor.matmul(po, lhsT=at[:, cc, :],
                                     rhs=wo_sb[:, cc * HPC + h2, :],
                                     start=(h2 == 0 and cc == 0),
                                     stop=(h2 == HPC - 1 and cc == NCORES - 1))
                if oi + 2 < len(order):
                    at0_load(*order[oi + 2])
            for qc2 in range(4):
                ot = stg_pool.tile([128, HPC * D], F32, tag="ot")
                nc.scalar.copy(out=ot, in_=pos[qc2])
                nc.sync.dma_start(out=out_ext[qc2 * 128:(qc2 + 1) * 128, :],
                                  in_=ot)

        # ---- main sequence ----
        cg_group(0)
        nc.sync.dma_start(out=msk_sb[:], in_=mskd[:, :])
        for g in range(1, 6):
            cg_group(g)
        nc.scalar.dma_start(out=wq_sb[:], in_=wq3[:, :, :])
        for g in range(6, 16):
            cg_group(g)
        # qg group 0 (q chunks 0,1 / kv 32,33) runs LAST so its deferred norm
        # can flush inside the first attention stream (which needs q chunks
        # 12-15 and all kv, but touches kv>=32 only from its 9th quad on).
        qg_group(1)
        qg_group(2)
        nc.sync.dma_start(out=wo_sb[:], in_=wo3[:, :, :])
        for g in (3, 4, 5, 6, 7, 0):
            qg_group(g)

        attn_stream(3, 0, split_at=32)
        attn_stream(3, 1)
        attn_stream(3, 2)
        attn_stream(3, 3)
        for h in range(HPC):
            attn_stream(2, h)
        oproj_block(3)
        for h in range(HPC):
            attn_stream(1, h)
        oproj_block(2)
        for h in range(HPC):
            attn_stream(0, h)
        flush_finalize()  # fires AG(0,3) immediately; oproj(1) hides it
        oproj_block(1)
        oproj_block0()

    nc.compile()
    return nc


def _tile_hid(a):
    """[HID, C] -> [128, NHD, C] with (p, k, c) = a[k*128+p, c]."""
    return np.ascontiguousarray(
        a.reshape(NHD, 128, a.shape[1]).transpose(1, 0, 2)).astype(bfloat16)


def _host_prep(context, query, w_qkv, w_o, q_norm_w, k_norm_w):
    context = np.asarray(context, dtype=np.float32)
    query = np.asarray(query, dtype=np.float32)
    w_qkv = np.asarray(w_qkv, dtype=np.float32)
    w_o = np.asarray(w_o, dtype=np.float32)
    q_norm_w = np.asarray(q_norm_w, dtype=np.float32)
    k_norm_w = np.asarray(k_norm_w, dtype=np.float32)

    ck = np.concatenate([context, query], axis=0)  # [KV, HID]
    ckT = np.ascontiguousarray(ck.T)  # [HID, KV] f32
    ckT3 = np.ascontiguousarray(
        ckT.reshape(NHD, 128, KV).transpose(1, 0, 2)).astype(bfloat16)

    wq = w_qkv[:, :H * D]
    wk = w_qkv[:, H * D:H * D + KVH * D]
    wv = w_qkv[:, H * D + KVH * D:]

    half = D // 2
    inv_freq = (1.0 / (THETA ** (np.arange(0, half, dtype=np.float32) / half))
                ).astype(np.float32)
    pos = np.arange(KV, dtype=np.float32)
    freqs = pos[:, None] * inv_freq[None, :]
    cosf, sinf = np.cos(freqs), np.sin(freqs)  # [KV, 64]

    def cs4(w):
        # [cos*w1 | sin*w2 | cos*w2 | sin*w1], w1 = w[:64], w2 = w[64:]
        return np.concatenate([cosf * w[None, :half], sinf * w[None, half:],
                               cosf * w[None, half:], sinf * w[None, :half]],
                              axis=1).astype(np.float32)  # [KV, 2D]

    cs4q = cs4(q_norm_w)[CTX:]  # [QL, 2D]
    cs4k = cs4(k_norm_w)  # [KV, 2D]
    csq_t = np.ascontiguousarray(
        cs4q.reshape(NQC, 128, 2 * D).transpose(1, 0, 2)).astype(bfloat16)
    csk_t = np.ascontiguousarray(
        cs4k.reshape(NKV, 128, 2 * D).transpose(1, 0, 2)).astype(bfloat16)

    p = np.arange(128)[:, None]
    t = np.arange(896)[None, :]
    msk = np.where(p <= t - 384, 0.0, MASKVAL).astype(bfloat16)  # [128, 896]

    in_maps = []
    for c in range(NCORES):
        in_maps.append({
            "ckT3": ckT3,
            "wq3": _tile_hid(wq[:, c * HPC * D:(c + 1) * HPC * D]),
            "wkv3": _tile_hid(np.concatenate(
                [wk[:, c * D:(c + 1) * D], wv[:, c * D:(c + 1) * D]], axis=1)),
            "wo3": _tile_hid(w_o[:, c * HPC * D:(c + 1) * HPC * D]),
            "csq": csq_t,
            "csk": csk_t,
            "msk": msk,
        })
    return in_maps


def kernel(context, query, w_qkv, w_o, q_norm_w, k_norm_w, **kw):
    if "nc" not in _STATE:
        _STATE["nc"] = _build()
    nc = _STATE["nc"]
    in_maps = _host_prep(context, query, w_qkv, w_o, q_norm_w, k_norm_w)
    res = run_bass_kernel_spmd(nc, in_maps, list(range(NCORES)), **kw)
    out = np.concatenate([np.asarray(res.results[c]["out"]) for c in range(NCORES)],
                         axis=1)
    if kw:
        return out.astype(np.float32), res
    return out.astype(np.float32)


# revision 18
# speedup vs baseline: 1.2321x; 1.2321x over previous
"""DFlash Qwen3 cross-attention on 8 TRN2 NeuronCores (v3).

Sharding: tensor-parallel over heads. Core c owns KV head c (KVH=8) and the
4 query heads 4c..4c+3 of its GQA group. Each core computes its heads'
QKV projections, per-head RMSNorm + RoPE, causal attention; per q-tile j the
normalized attention outputs (transposed, [4*D, 512]) are AllGathered so
every core holds attn^T for all 32 heads; each core computes a 512-column
slice of o_proj and the host concatenates the 8 slices.

v3 structure (vs v2, driven by the NTFF profile of v2):
- phase order: context proj (16 groups) -> query proj (groups 1..7 then 0)
  -> attention j=3,2,1,0 -> o_proj j' trailing one attention block. The last
  attention block is the cheapest (j=0), and o_proj(1) hides AG(0) latency.
- bubble-free attention quads: both score-pair matmuls emitted before the
  exps, AV matmuls after, so the PE never waits a full exp latency mid-quad.
- stream finalize (denominator matmuls, reciprocal, normalize, AllGather
  payload write) deferred into the NEXT stream after its first quad: kills
  the ~2.8us PE stall + HAM re-throttle at every stream boundary.
- per-j single AllGather with a [128, 4(qc), 4(h), 128] payload so o_proj
  reads back [128, 8c, 4h, 128] tiles as 1KB-contiguous segments, prefetched
  double-buffered on the idle sync DMA queue.
- RMSNorm rsqrt fully on DVE (quake bit-trick + 1 Newton step): the ACT
  engine runs exp only -- no ACT table swaps (v2 paid 27 x 1.3us mid-attention).
- wkv weight DMA split in 4 + non-critical loads (wq/wo/msk) deferred past
  the startup barrier window so the first matmul issues ~40us earlier.
"""

from contextlib import ExitStack

import numpy as np
from ml_dtypes import bfloat16

import concourse.bass as bass
import concourse.bass_isa as bass_isa
import concourse.mybir as mybir
import concourse.tile as tile
from concourse import bacc
from concourse.bass_utils import run_bass_kernel_spmd
from concourse.masks import make_identity

H = 32
KVH = 8
D = 128
HID = 4096
CTX = 4096
QL = 2048
KV = CTX + QL  # 6144
NCORES = 8
HPC = H // NCORES  # 4 q heads per core
THETA = 1000000.0
EPS = 1e-6
SCALE = float(D) ** -0.5

NHD = HID // 128  # 32 contraction chunks
NKV = KV // 128  # 48 kv chunks
NQC = QL // 128  # 16 q row chunks
NQJ = QL // 512  # 4 q column tiles for attention
NCTX = CTX // 128  # 32 context kv chunks
MASKVAL = -1e6
MAGIC = 0x5F3759DF  # fast inverse sqrt seed

F32 = mybir.dt.float32
BF16 = mybir.dt.bfloat16
I32 = mybir.dt.int32
ALU = mybir.AluOpType

_STATE = {}


def _build():
    nc = bacc.Bacc()

    ckT3 = nc.declare_dram_parameter("ckT3", [128, NHD, KV], BF16, isOutput=False)
    wq3 = nc.declare_dram_parameter("wq3", [128, NHD, HPC * D], BF16, isOutput=False)
    wkv3 = nc.declare_dram_parameter("wkv3", [128, NHD, 2 * D], BF16, isOutput=False)
    wo3 = nc.declare_dram_parameter("wo3", [128, NHD, HPC * D], BF16, isOutput=False)
    csq = nc.declare_dram_parameter("csq", [128, NQC, 2 * D], BF16, isOutput=False)
    csk = nc.declare_dram_parameter("csk", [128, NKV, 2 * D], BF16, isOutput=False)
    mskd = nc.declare_dram_parameter("msk", [128, 896], BF16, isOutput=False)
    out_ext = nc.declare_dram_parameter("out", [QL, HPC * D], F32, isOutput=True)

    warm_in = nc.dram_tensor("warm_in", [128, 8], BF16)
    warm_out = nc.dram_tensor("warm_out", [NCORES * 128, 8], BF16, addr_space="Shared")
    # AllGather payload per j: [128 d-part, 4 qc, 4 h, 128 q] so the o_proj
    # readback for one qc is 1KB-contiguous per core block. j=0 (the LAST
    # attention block) gathers per head instead so only a 1MB collective and
    # the h=3 quarter of o_proj(0) remain on the critical tail.
    ag_ins = [nc.dram_tensor(f"ag_in{j}", [128, 4, HPC, 128], BF16)
              for j in range(1, NQJ)]
    ag_outs = [nc.dram_tensor(f"ag_out{j}", [NCORES * 128, 4, HPC, 128], BF16,
                              addr_space="Shared") for j in range(1, NQJ)]
    ag0_ins = [nc.dram_tensor(f"ag0_in{h}", [128, 4, 128], BF16)
               for h in range(HPC)]
    ag0_outs = [nc.dram_tensor(f"ag0_out{h}", [NCORES * 128, 4, 128], BF16,
                               addr_space="Shared") for h in range(HPC)]

    with tile.TileContext(nc) as tc, ExitStack() as ctx:
        singles = ctx.enter_context(tc.tile_pool(name="singles", bufs=1))
        ckv_pool = ctx.enter_context(tc.tile_pool(name="ckv", bufs=3))
        evac_pool = ctx.enter_context(tc.tile_pool(name="evac", bufs=3))
        work_pool = ctx.enter_context(tc.tile_pool(name="work", bufs=2))
        pt_pool = ctx.enter_context(tc.tile_pool(name="ptp", bufs=3))
        saccs_pool = ctx.enter_context(tc.tile_pool(name="sac", bufs=2))
        stg_pool = ctx.enter_context(tc.tile_pool(name="stg", bufs=2))
        at_pool = ctx.enter_context(tc.tile_pool(name="atp", bufs=2))
        # PSUM: 4 x 1-bank accumulators + 2 x 2-bank score tiles = 8 banks
        p1 = ctx.enter_context(tc.tile_pool(name="p1", bufs=4, space="PSUM"))
        p2 = ctx.enter_context(tc.tile_pool(name="p2", bufs=2, space="PSUM"))

        # ---- critical-path DMAs first: wkv (split x4) + first ck chunks ----
        wkv_sb = singles.tile([128, NHD, 2 * D], BF16)
        for q4 in range(4):
            nc.scalar.dma_start(out=wkv_sb[:, q4 * 8:(q4 + 1) * 8, :],
                                in_=wkv3[:, q4 * 8:(q4 + 1) * 8, :])

        def load_ckv(c0):
            """Stream ckT3[:, :, c0:c0+256] in two k-halves (sync queue, so
            the warmup collective trigger on gpsimd can't block them)."""
            ca = ckv_pool.tile([128, NHD // 2, 256], BF16, tag="ckv")
            nc.sync.dma_start(out=ca, in_=ckT3[:, 0:NHD // 2, c0:c0 + 256])
            cb = ckv_pool.tile([128, NHD // 2, 256], BF16, tag="ckv")
            nc.sync.dma_start(out=cb, in_=ckT3[:, NHD // 2:NHD, c0:c0 + 256])

            def sl(k, c):
                t = ca if k < NHD // 2 else cb
                return t[:, k % (NHD // 2), c * 128:(c + 1) * 128]
            return sl

        # ---- warmup collective: pay the first-collective handshake now ----
        wtile = singles.tile([128, 8], BF16)
        nc.vector.memset(wtile, 0.0)
        nc.gpsimd.dma_start(out=warm_in[:, :], in_=wtile)
        nc.gpsimd.collective_compute(
            "AllGather", ALU.bypass, ins=[warm_in[:, :]], outs=[warm_out[:, :]],
            replica_groups=[list(range(NCORES))])

        # ---- constants ----
        identb = singles.tile([128, 128], BF16)
        make_identity(nc, identb)
        ones_bf = singles.tile([128, 128], BF16)
        nc.vector.memset(ones_bf, 1.0)
        zbias = singles.tile([128, 1], F32)
        nc.vector.memset(zbias, 0.0)
        magict = singles.tile([128, 4], I32)
        nc.vector.memset(magict, MAGIC)

        wq_sb = singles.tile([128, NHD, HPC * D], BF16)  # loaded mid-cg
        # compact causal mask: msk_sb[p, 384-128i + q] = 0 if 128i+p <= q
        # else MASKVAL, so chunk i uses the slice [384-128i : 896-128i].
        msk_sb = singles.tile([128, 896], BF16)  # loaded after cg0
        wo_sb = singles.tile([128, NHD, HPC * D], BF16)  # loaded mid-qg

        qT_sb = singles.tile([128, HPC, QL], BF16)  # Q^T per head: [d, h, q]
        kT_sb = singles.tile([128, KV], BF16)  # K^T: [d, kv]
        v_sb = singles.tile([128, NKV, D], BF16)  # V: [kv%128, r, d]

        def rsqrt_sum(rr, ssum, n, tg):
            """rr = 1/sqrt(ssum/D + eps), all on DVE (no ACT table swap)."""
            x = work_pool.tile([128, n], F32, tag=f"rx{tg}")
            nc.vector.tensor_scalar(out=x, in0=ssum, scalar1=1.0 / D,
                                    scalar2=EPS, op0=ALU.mult, op1=ALU.add)
            yi = work_pool.tile([128, n], I32, tag=f"ry{tg}")
            nc.vector.tensor_scalar(out=yi, in0=x[:, :].bitcast(I32), scalar1=1,
                                    scalar2=None, op0=ALU.logical_shift_right)
            nc.vector.tensor_tensor(out=yi, in0=magict[:, 0:n], in1=yi,
                                    op=ALU.subtract)
            y = yi[:, :].bitcast(F32)
            t1 = work_pool.tile([128, n], F32, tag=f"rt{tg}")
            nc.vector.tensor_tensor(out=t1, in0=y, in1=y, op=ALU.mult)
            nc.vector.tensor_tensor(out=t1, in0=t1, in1=x, op=ALU.mult)
            nc.vector.tensor_scalar(out=t1, in0=t1, scalar1=-0.5, scalar2=1.5,
                                    op0=ALU.mult, op1=ALU.add)
            nc.vector.tensor_tensor(out=rr, in0=t1, in1=y, op=ALU.mult)

        def rope(ro, xn, cs, t1):
            """ro = rotate-half(xn) with cs = [cw1|sw2|cw2|sw1] slices."""
            hf = D // 2
            nc.vector.tensor_mul(ro[:, :, 0:hf], xn[:, :, 0:hf], cs[0])
            nc.vector.tensor_mul(t1, xn[:, :, hf:D], cs[1])
            nc.vector.tensor_sub(ro[:, :, 0:hf], ro[:, :, 0:hf], t1)
            nc.vector.tensor_mul(ro[:, :, hf:D], xn[:, :, hf:D], cs[2])
            nc.vector.tensor_mul(t1, xn[:, :, 0:hf], cs[3])
            nc.vector.tensor_add(ro[:, :, hf:D], ro[:, :, hf:D], t1)

        def qnorm(qe, qc, csqp, ci):
            """qe: [128, 4, 128] bf16 SBUF evac of the q projection.
            norm+rope -> 4 transposes -> qT_sb."""
            sq = work_pool.tile([128, HPC, D], BF16, tag="qsq")
            nc.vector.tensor_mul(sq, qe, qe)
            ssum = work_pool.tile([128, HPC], F32, tag="qssum")
            nc.vector.tensor_reduce(ssum, sq, axis=mybir.AxisListType.X, op=ALU.add)
            rr = work_pool.tile([128, HPC], F32, tag="qrr")
            rsqrt_sum(rr, ssum, HPC, "q")
            qn = work_pool.tile([128, HPC, D], BF16, tag="qn")
            for h in range(HPC):
                nc.vector.tensor_scalar_mul(out=qn[:, h, :], in0=qe[:, h, :],
                                            scalar1=rr[:, h:h + 1])
            hf = D // 2
            ro = work_pool.tile([128, HPC, D], BF16, tag="qro")
            t1 = work_pool.tile([128, HPC, hf], BF16, tag="qt1")
            for h in range(HPC):
                cs = [csqp[:, ci:ci + 1, s * hf:(s + 1) * hf] for s in range(4)]
                rope(ro[:, h:h + 1, :], qn[:, h:h + 1, :], cs, t1[:, 0:1, :])
            for h in range(HPC):
                tp = p1.tile([128, 128], BF16, tag="acc", name=f"tpq{qc}_{h}")
                nc.tensor.transpose(tp, ro[:, h, :], identb)
                nc.scalar.copy(out=qT_sb[:, h, qc * 128:(qc + 1) * 128], in_=tp)

        def knorm_pair(ke, r0, cskp):
            """ke: [128, 2, 2D] bf16 SBUF evac of the K|V projection pair."""
            sq = work_pool.tile([128, 2, D], BF16, tag="ksq")
            nc.vector.tensor_mul(sq, ke[:, :, 0:D], ke[:, :, 0:D])
            ssum = work_pool.tile([128, 2], F32, tag="kssum")
            nc.vector.tensor_reduce(ssum, sq, axis=mybir.AxisListType.X, op=ALU.add)
            rr = work_pool.tile([128, 2], F32, tag="krr")
            rsqrt_sum(rr, ssum, 2, "k")
            kn = work_pool.tile([128, 2, D], BF16, tag="kn")
            for c in range(2):
                nc.vector.tensor_scalar_mul(out=kn[:, c, :], in0=ke[:, c, 0:D],
                                            scalar1=rr[:, c:c + 1])
            hf = D // 2
            cs = [cskp[:, :, s * hf:(s + 1) * hf] for s in range(4)]
            ro = work_pool.tile([128, 2, D], BF16, tag="kro")
            t1 = work_pool.tile([128, 2, hf], BF16, tag="kt1")
            rope(ro, kn, cs, t1)
            for c in range(2):
                tp = p1.tile([128, 128], BF16, tag="acc", name=f"tpk{r0}_{c}")
                nc.tensor.transpose(tp, ro[:, c, :], identb)
                nc.scalar.copy(out=kT_sb[:, (r0 + c) * 128:(r0 + c + 1) * 128], in_=tp)
            nc.vector.tensor_copy(out=v_sb[:, r0:r0 + 2, :], in_=ke[:, :, D:2 * D])

        # Deferred-norm pipeline: each projection group's norm+rope+PE
        # transposes are emitted AFTER the NEXT group's matmuls so the DVE
        # chain hides behind tensor work.
        pending = []

        def flush_pending():
            while pending:
                pending.pop(0)()

        def cg_group(g):
            """Context projection: kv chunks 2g, 2g+1."""
            sl = load_ckv(g * 256)
            cskp = evac_pool.tile([128, 2, 2 * D], BF16, tag="cskp")
            nc.scalar.dma_start(out=cskp, in_=csk[:, 2 * g:2 * g + 2, :])
            pk = p1.tile([128, 2, 2 * D], F32, tag="acc", name=f"cgk{g}")
            for k in range(NHD):
                for c in range(2):
                    nc.tensor.matmul(pk[:, c, :], lhsT=sl(k, c),
                                     rhs=wkv_sb[:, k, :],
                                     start=(k == 0 and c == 0),
                                     stop=(k == NHD - 1))
            ke = evac_pool.tile([128, 2, 2 * D], BF16, tag="ke")
            nc.scalar.copy(out=ke[:], in_=pk)
            flush_pending()
            pending.append(lambda ke=ke, r0=2 * g, cskp=cskp: knorm_pair(ke, r0, cskp))

        def qg_group(g):
            """Query-row projection: kv chunks 32+2g, 33+2g (q chunks 2g,
            2g+1): shared stationary feeds both wkv and wq."""
            sl = load_ckv(CTX + g * 256)
            cskp = evac_pool.tile([128, 2, 2 * D], BF16, tag="cskp")
            nc.scalar.dma_start(out=cskp, in_=csk[:, NCTX + 2 * g:NCTX + 2 * g + 2, :])
            csqp = evac_pool.tile([128, 2, 2 * D], BF16, tag="csqp")
            nc.scalar.dma_start(out=csqp, in_=csq[:, 2 * g:2 * g + 2, :])
            pk = p1.tile([128, 2, 2 * D], F32, tag="acc", name=f"qgk{g}")
            pq = [p1.tile([128, HPC * D], F32, tag="acc", name=f"qgq{g}_{c}")
                  for c in range(2)]
            for k in range(NHD):
                for c in range(2):
                    st = sl(k, c)
                    nc.tensor.matmul(pk[:, c, :], lhsT=st, rhs=wkv_sb[:, k, :],
                                     start=(k == 0 and c == 0),
                                     stop=(k == NHD - 1))
                    nc.tensor.matmul(pq[c], lhsT=st, rhs=wq_sb[:, k, :],
                                     start=(k == 0), stop=(k == NHD - 1))
            ke = evac_pool.tile([128, 2, 2 * D], BF16, tag="ke")
            nc.scalar.copy(out=ke[:], in_=pk)
            qes = []
            for c in range(2):
                qe = evac_pool.tile([128, HPC, D], BF16, tag="qe")
                nc.scalar.copy(out=qe[:], in_=pq[c])
                qes.append(qe)
            flush_pending()

            def norm(ke=ke, qes=qes, csqp=csqp, cskp=cskp, g=g):
                knorm_pair(ke, NCTX + 2 * g, cskp)
                for c in range(2):
                    qnorm(qes[c], 2 * g + c, csqp, c)
            pending.append(norm)

        # ---- attention ----
        # finalize (den matmuls, reciprocal, normalize, AG payload write) of
        # the previous stream is deferred and emitted after the next block of
        # dense PE work has started.
        finalize_pending = []

        def flush_finalize():
            while finalize_pending:
                finalize_pending.pop(0)()

        def make_finalize(j, h, o_acc, saccs):
            def fin():
                den = p1.tile([128, 512], F32, tag="acc", name=f"den{j}_{h}")
                for s in range(4):
                    nc.tensor.matmul(den, lhsT=ones_bf,
                                     rhs=saccs[:, s * 512:(s + 1) * 512],
                                     start=(s == 0), stop=(s == 3))
                pr = work_pool.tile([128, 512], F32, tag="dps")
                nc.vector.reciprocal_approx_fast(out=pr, in_=den)
                s3 = stg_pool.tile([128, 512], BF16, tag="stg")
                nc.vector.tensor_mul(s3, o_acc, pr)
                s3v = s3[:, :].rearrange("p (c q) -> p c q", c=4)
                if j == 0:
                    nc.gpsimd.dma_start(out=ag0_ins[h][:, :, :], in_=s3v)
                    nc.gpsimd.collective_compute(
                        "AllGather", ALU.bypass, ins=[ag0_ins[h][:, :, :]],
                        outs=[ag0_outs[h][:, :, :]],
                        replica_groups=[list(range(NCORES))])
                else:
                    nc.gpsimd.dma_start(out=ag_ins[j - 1][:, :, h, :], in_=s3v)
                    if h == HPC - 1:
                        nc.gpsimd.collective_compute(
                            "AllGather", ALU.bypass,
                            ins=[ag_ins[j - 1][:, :, :, :]],
                            outs=[ag_outs[j - 1][:, :, :, :]],
                            replica_groups=[list(range(NCORES))])
            return fin

        def attn_stream(j, h, split_at=None):
            """One (j,h) attention stream over all its kv chunks.
            split_at: optional chunk index at which to flush the projection
            norm pipeline (used once, to hide the last qg group's norm)."""
            r_list = list(range(NCTX + 4 + 4 * j))
            nq = len(r_list) // 4
            qTj = qT_sb[:, h, j * 512:(j + 1) * 512]
            o_acc = p1.tile([128, 512], F32, tag="acc", name=f"o{j}_{h}")
            saccs = saccs_pool.tile([128, 4 * 512], BF16, tag="sacc")
            for qd in range(nq):
                if split_at is not None and qd * 4 == split_at:
                    flush_pending()
                ptw = pt_pool.tile([128, 4 * 512], BF16, tag="ptw")
                sts = []
                for pi in range(2):
                    rA = r_list[qd * 4 + 2 * pi]
                    rB = r_list[qd * 4 + 2 * pi + 1]
                    st = p2.tile([128, 1024], F32, tag="st")
                    nc.tensor.matmul(st[:, 0:512],
                                     lhsT=kT_sb[:, rA * 128:(rA + 1) * 128],
                                     rhs=qTj, start=True, stop=True)
                    nc.tensor.matmul(st[:, 512:1024],
                                     lhsT=kT_sb[:, rB * 128:(rB + 1) * 128],
                                     rhs=qTj, start=True, stop=True)
                    for idx, r in ((0, rA), (1, rB)):
                        i = r - NCTX - 4 * j
                        if 0 <= i <= 3:
                            nc.vector.tensor_add(
                                st[:, idx * 512:(idx + 1) * 512],
                                st[:, idx * 512:(idx + 1) * 512],
                                msk_sb[:, 384 - 128 * i:896 - 128 * i])
                    sts.append(st)
                for pi in range(2):
                    nc.scalar.activation(out=ptw[:, pi * 1024:(pi + 1) * 1024],
                                         in_=sts[pi][:],
                                         func=mybir.ActivationFunctionType.Exp,
                                         bias=zbias, scale=SCALE)
                for slot in range(4):
                    r = r_list[qd * 4 + slot]
                    nc.tensor.matmul(
                        o_acc, lhsT=v_sb[:, r, :],
                        rhs=ptw[:, slot * 512:(slot + 1) * 512],
                        start=(qd == 0 and slot == 0),
                        stop=(qd == nq - 1 and slot == 3))
                if qd == 0:
                    flush_finalize()
                    nc.vector.tensor_copy(out=saccs, in_=ptw)
                else:
                    nc.vector.tensor_add(saccs, saccs, ptw)
            finalize_pending.append(make_finalize(j, h, o_acc, saccs))

        def oproj_block(jq):
            """o_proj for q tile jq (jq >= 1) from the AllGathered attn^T.
            Tiles load per (qc, 4-core half) as 1KB-contiguous segments."""
            ats = {}

            def at_load(hv):
                qc2, lo = hv // 2, (hv % 2) * 4
                at = at_pool.tile([128, 4, HPC, 128], BF16, tag="at")
                nc.sync.dma_start(
                    out=at,
                    in_=ag_outs[jq - 1][lo * 128:(lo + 4) * 128, qc2, :, :]
                    .rearrange("(c p) h q -> p c h q", p=128))
                ats[hv] = at

            at_load(0)
            at_load(1)
            for qc2 in range(4):
                qc = 4 * jq + qc2
                po = p1.tile([128, HPC * D], F32, tag="acc", name=f"po{qc}")
                for half in range(2):
                    at = ats.pop(qc2 * 2 + half)
                    for c4 in range(4):
                        for h2 in range(HPC):
                            gg = (half * 4 + c4) * HPC + h2
                            nc.tensor.matmul(po, lhsT=at[:, c4, h2, :],
                                             rhs=wo_sb[:, gg, :],
                                             start=(gg == 0), stop=(gg == H - 1))
                    if qc2 == 0 and half == 0:
                        flush_finalize()
                    if qc2 * 2 + half + 2 < 8:
                        at_load(qc2 * 2 + half + 2)
                ot = stg_pool.tile([128, HPC * D], F32, tag="ot")
                nc.scalar.copy(out=ot, in_=po)
                nc.sync.dma_start(out=out_ext[qc * 128:(qc + 1) * 128, :], in_=ot)

        def oproj_block0():
            """o_proj for q tile 0, head-major: the h2<3 contraction chunks
            run while AG(0, h=3) is still in flight, so only the last-quarter
            matmuls trail the final collective."""
            pos = [p1.tile([128, HPC * D], F32, tag="acc", name=f"po0_{qc2}")
                   for qc2 in range(4)]
            ats = {}

            def at0_load(h2, qc2):
                at = at_pool.tile([128, NCORES, 128], BF16, tag="at")
                nc.sync.dma_start(
                    out=at,
                    in_=ag0_outs[h2][:, qc2, :].rearrange("(c p) q -> p c q",
                                                          p=128))
                ats[(h2, qc2)] = at

            at0_load(0, 0)
            at0_load(0, 1)
            order = [(h2, qc2) for h2 in range(HPC) for qc2 in range(4)]
            for oi, (h2, qc2) in enumerate(order):
                at = ats.pop((h2, qc2))
                po = pos[qc2]
                for cc in range(NCORES):
                    nc.tensor.matmul(po, lhsT=at[:, cc, :],
                                     rhs=wo_sb[:, cc * HPC + h2, :],
                                     start=(h2 == 0 and cc == 0),
                                     stop=(h2 == HPC - 1 and cc == NCORES - 1))
                if oi + 2 < len(order):
                    at0_load(*order[oi + 2])
            for qc2 in range(4):
                ot = stg_pool.tile([128, HPC * D], F32, tag="ot")
                nc.scalar.copy(out=ot, in_=pos[qc2])
                nc.sync.dma_start(out=out_ext[qc2 * 128:(qc2 + 1) * 128, :],
                                  in_=ot)

        # ---- main sequence ----
        cg_group(0)
        nc.sync.dma_start(out=msk_sb[:], in_=mskd[:, :])
        for g in range(1, 6):
            cg_group(g)
        nc.scalar.dma_start(out=wq_sb[:], in_=wq3[:, :, :])
        for g in range(6, 16):
            cg_group(g)
        # qg group 0 (q chunks 0,1 / kv 32,33) runs LAST so its deferred norm
        # can flush inside the first attention stream (which needs q chunks
        # 12-15 and all kv, but touches kv>=32 only from its 9th quad on).
        qg_group(1)
        qg_group(2)
        nc.sync.dma_start(out=wo_sb[:], in_=wo3[:, :, :])
        for g in (3, 4, 5, 6, 7, 0):
            qg_group(g)

        attn_stream(3, 0, split_at=32)
        attn_stream(3, 1)
        attn_stream(3, 2)
        attn_stream(3, 3)
        for h in range(HPC):
            attn_stream(2, h)
        oproj_block(3)
        for h in range(HPC):
            attn_stream(1, h)
        oproj_block(2)
        for h in range(HPC):
            attn_stream(0, h)
        flush_finalize()  # fires AG(0,3) immediately; oproj(1) hides it
        oproj_block(1)
        oproj_block0()

    nc.compile()
    return nc


def _tile_hid(a):
    """[HID, C] -> [128, NHD, C] with (p, k, c) = a[k*128+p, c]."""
    return np.ascontiguousarray(
        a.reshape(NHD, 128, a.shape[1]).transpose(1, 0, 2)).astype(bfloat16)


def _host_prep(context, query, w_qkv, w_o, q_norm_w, k_norm_w):
    context = np.asarray(context, dtype=np.float32)
    query = np.asarray(query, dtype=np.float32)
    w_qkv = np.asarray(w_qkv, dtype=np.float32)
    w_o = np.asarray(w_o, dtype=np.float32)
    q_norm_w = np.asarray(q_norm_w, dtype=np.float32)
    k_norm_w = np.asarray(k_norm_w, dtype=np.float32)

    ck = np.concatenate([context, query], axis=0)  # [KV, HID]
    ckT = np.ascontiguousarray(ck.T)  # [HID, KV] f32
    ckT3 = np.ascontiguousarray(
        ckT.reshape(NHD, 128, KV).transpose(1, 0, 2)).astype(bfloat16)

    wq = w_qkv[:, :H * D]
    wk = w_qkv[:, H * D:H * D + KVH * D]
    wv = w_qkv[:, H * D + KVH * D:]

    half = D // 2
    inv_freq = (1.0 / (THETA ** (np.arange(0, half, dtype=np.float32) / half))
                ).astype(np.float32)
    pos = np.arange(KV, dtype=np.float32)
    freqs = pos[:, None] * inv_freq[None, :]
    cosf, sinf = np.cos(freqs), np.sin(freqs)  # [KV, 64]

    def cs4(w):
        # [cos*w1 | sin*w2 | cos*w2 | sin*w1], w1 = w[:64], w2 = w[64:]
        return np.concatenate([cosf * w[None, :half], sinf * w[None, half:],
                               cosf * w[None, half:], sinf * w[None, :half]],
                              axis=1).astype(np.float32)  # [KV, 2D]

    cs4q = cs4(q_norm_w)[CTX:]  # [QL, 2D]
    cs4k = cs4(k_norm_w)  # [KV, 2D]
    csq_t = np.ascontiguousarray(
        cs4q.reshape(NQC, 128, 2 * D).transpose(1, 0, 2)).astype(bfloat16)
    csk_t = np.ascontiguousarray(
        cs4k.reshape(NKV, 128, 2 * D).transpose(1, 0, 2)).astype(bfloat16)

    p = np.arange(128)[:, None]
    t = np.arange(896)[None, :]
    msk = np.where(p <= t - 384, 0.0, MASKVAL).astype(bfloat16)  # [128, 896]

    in_maps = []
    for c in range(NCORES):
        in_maps.append({
            "ckT3": ckT3,
            "wq3": _tile_hid(wq[:, c * HPC * D:(c + 1) * HPC * D]),
            "wkv3": _tile_hid(np.concatenate(
                [wk[:, c * D:(c + 1) * D], wv[:, c * D:(c + 1) * D]], axis=1)),
            "wo3": _tile_hid(w_o[:, c * HPC * D:(c + 1) * HPC * D]),
            "csq": csq_t,
            "csk": csk_t,
            "msk": msk,
        })
    return in_maps


def kernel(context, query, w_qkv, w_o, q_norm_w, k_norm_w, **kw):
    if "nc" not in _STATE:
        _STATE["nc"] = _build()
    nc = _STATE["nc"]
    in_maps = _host_prep(context, query, w_qkv, w_o, q_norm_w, k_norm_w)
    res = run_bass_kernel_spmd(nc, in_maps, list(range(NCORES)), **kw)
    out = np.concatenate([np.asarray(res.results[c]["out"]) for c in range(NCORES)],
                         axis=1)
    if kw:
        return out.astype(np.float32), res
    return out.astype(np.float32)


# revision 23
# speedup vs baseline: 1.3075x; 1.0612x over previous
"""DFlash Qwen3 cross-attention on 8 TRN2 NeuronCores (v3).

Sharding: tensor-parallel over heads. Core c owns KV head c (KVH=8) and the
4 query heads 4c..4c+3 of its GQA group. Each core computes its heads'
QKV projections, per-head RMSNorm + RoPE, causal attention; per q-tile j the
normalized attention outputs (transposed, [4*D, 512]) are AllGathered so
every core holds attn^T for all 32 heads; each core computes a 512-column
slice of o_proj and the host concatenates the 8 slices.

v3 structure (vs v2, driven by the NTFF profile of v2):
- phase order: context proj (16 groups) -> query proj (groups 1..7 then 0)
  -> attention j=3,2,1,0 -> o_proj j' trailing one attention block. The last
  attention block is the cheapest (j=0), and o_proj(1) hides AG(0) latency.
- bubble-free attention quads: both score-pair matmuls emitted before the
  exps, AV matmuls after, so the PE never waits a full exp latency mid-quad.
- stream finalize (denominator matmuls, reciprocal, normalize, AllGather
  payload write) deferred into the NEXT stream after its first quad: kills
  the ~2.8us PE stall + HAM re-throttle at every stream boundary.
- per-j single AllGather with a [128, 4(qc), 4(h), 128] payload so o_proj
  reads back [128, 8c, 4h, 128] tiles as 1KB-contiguous segments, prefetched
  double-buffered on the idle sync DMA queue.
- RMSNorm rsqrt fully on DVE (quake bit-trick + 1 Newton step): the ACT
  engine runs exp only -- no ACT table swaps (v2 paid 27 x 1.3us mid-attention).
- wkv weight DMA split in 4 + non-critical loads (wq/wo/msk) deferred past
  the startup barrier window so the first matmul issues ~40us earlier.
"""

from contextlib import ExitStack

import numpy as np
from ml_dtypes import bfloat16

import concourse.bass as bass
import concourse.bass_isa as bass_isa
import concourse.mybir as mybir
import concourse.tile as tile
from concourse import bacc
from concourse.bass_utils import run_bass_kernel_spmd
from concourse.masks import make_identity

H = 32
KVH = 8
D = 128
HID = 4096
CTX = 4096
QL = 2048
KV = CTX + QL  # 6144
NCORES = 8
HPC = H // NCORES  # 4 q heads per core
THETA = 1000000.0
EPS = 1e-6
SCALE = float(D) ** -0.5

NHD = HID // 128  # 32 contraction chunks
NKV = KV // 128  # 48 kv chunks
NQC = QL // 128  # 16 q row chunks
NQJ = QL // 512  # 4 q column tiles for attention
NCTX = CTX // 128  # 32 context kv chunks
MASKVAL = -1e6
MAGIC = 0x5F3759DF  # fast inverse sqrt seed

F32 = mybir.dt.float32
BF16 = mybir.dt.bfloat16
I32 = mybir.dt.int32
ALU = mybir.AluOpType

_STATE = {}


def _build():
    nc = bacc.Bacc()

    # ck^T in 256-kv-column blocks, [g][p][k][c] so one group load is an
    # 8KB-contiguous segment per partition (cheap descriptors, full HBM bw).
    ckT4 = nc.declare_dram_parameter("ckT4", [KV // 256, 128, NHD, 256], BF16,
                                     isOutput=False)
    wq3 = nc.declare_dram_parameter("wq3", [128, NHD, HPC * D], BF16, isOutput=False)
    wkv3 = nc.declare_dram_parameter("wkv3", [128, NHD, 2 * D], BF16, isOutput=False)
    wo3 = nc.declare_dram_parameter("wo3", [128, NHD, HPC * D], BF16, isOutput=False)
    csq = nc.declare_dram_parameter("csq", [128, NQC, 2 * D], BF16, isOutput=False)
    csk = nc.declare_dram_parameter("csk", [128, NKV, 2 * D], BF16, isOutput=False)
    mskd = nc.declare_dram_parameter("msk", [128, 896], BF16, isOutput=False)
    out_ext = nc.declare_dram_parameter("out", [QL, HPC * D], F32, isOutput=True)

    warm_in = nc.dram_tensor("warm_in", [128, 8], BF16)
    warm_out = nc.dram_tensor("warm_out", [NCORES * 128, 8], BF16, addr_space="Shared")
    # AllGather payload per j: [128 d-part, 4 qc, 4 h, 128 q] so the o_proj
    # readback for one qc is 1KB-contiguous per core block. j=0 (the LAST
    # attention block) gathers per head instead so only a 1MB collective and
    # the h=3 quarter of o_proj(0) remain on the critical tail.
    ag_ins = [nc.dram_tensor(f"ag_in{j}", [128, 4, HPC, 128], BF16)
              for j in range(1, NQJ)]
    ag_outs = [nc.dram_tensor(f"ag_out{j}", [NCORES * 128, 4, HPC, 128], BF16,
                              addr_space="Shared") for j in range(1, NQJ)]
    ag0_ins = [nc.dram_tensor(f"ag0_in{h}", [128, 4, 128], BF16)
               for h in range(HPC)]
    ag0_outs = [nc.dram_tensor(f"ag0_out{h}", [NCORES * 128, 4, 128], BF16,
                               addr_space="Shared") for h in range(HPC)]

    with tile.TileContext(nc) as tc, ExitStack() as ctx:
        singles = ctx.enter_context(tc.tile_pool(name="singles", bufs=1))
        ckv_pool = ctx.enter_context(tc.tile_pool(name="ckv", bufs=3))
        evac_pool = ctx.enter_context(tc.tile_pool(name="evac", bufs=3))
        work_pool = ctx.enter_context(tc.tile_pool(name="work", bufs=2))
        pt_pool = ctx.enter_context(tc.tile_pool(name="ptp", bufs=3))
        saccs_pool = ctx.enter_context(tc.tile_pool(name="sac", bufs=2))
        stg_pool = ctx.enter_context(tc.tile_pool(name="stg", bufs=2))
        at_pool = ctx.enter_context(tc.tile_pool(name="atp", bufs=2))
        # PSUM: 4 x 1-bank accumulators + 2 x 2-bank score tiles = 8 banks
        p1 = ctx.enter_context(tc.tile_pool(name="p1", bufs=4, space="PSUM"))
        p2 = ctx.enter_context(tc.tile_pool(name="p2", bufs=2, space="PSUM"))

        # ---- critical-path DMAs first: wkv (split x4) + first ck chunks ----
        wkv_sb = singles.tile([128, NHD, 2 * D], BF16)
        for q4 in range(4):
            nc.scalar.dma_start(out=wkv_sb[:, q4 * 8:(q4 + 1) * 8, :],
                                in_=wkv3[:, q4 * 8:(q4 + 1) * 8, :])

        def load_ckv(c0):
            """Stream ck^T block c0//256 in two k-halves (sync queue, so
            the warmup collective trigger on gpsimd can't block them)."""
            g2 = c0 // 256
            ca = ckv_pool.tile([128, NHD // 2, 256], BF16, tag="ckv")
            nc.sync.dma_start(out=ca, in_=ckT4[g2, :, 0:NHD // 2, :])
            cb = ckv_pool.tile([128, NHD // 2, 256], BF16, tag="ckv")
            nc.sync.dma_start(out=cb, in_=ckT4[g2, :, NHD // 2:NHD, :])

            def sl(k, c):
                t = ca if k < NHD // 2 else cb
                return t[:, k % (NHD // 2), c * 128:(c + 1) * 128]
            return sl

        # ---- warmup collective: pay the first-collective handshake now ----
        wtile = singles.tile([128, 8], BF16)
        nc.vector.memset(wtile, 0.0)
        nc.gpsimd.dma_start(out=warm_in[:, :], in_=wtile)
        nc.gpsimd.collective_compute(
            "AllGather", ALU.bypass, ins=[warm_in[:, :]], outs=[warm_out[:, :]],
            replica_groups=[list(range(NCORES))])

        # ---- constants ----
        identb = singles.tile([128, 128], BF16)
        make_identity(nc, identb)
        ones_bf = singles.tile([128, 128], BF16)
        nc.vector.memset(ones_bf, 1.0)
        zbias = singles.tile([128, 1], F32)
        nc.vector.memset(zbias, 0.0)
        magict = singles.tile([128, 4], I32)
        nc.vector.memset(magict, MAGIC)

        wq_sb = singles.tile([128, NHD, HPC * D], BF16)  # loaded mid-cg
        # compact causal mask: msk_sb[p, 384-128i + q] = 0 if 128i+p <= q
        # else MASKVAL, so chunk i uses the slice [384-128i : 896-128i].
        msk_sb = singles.tile([128, 896], BF16)  # loaded after cg0
        wo_sb = singles.tile([128, NHD, HPC * D], BF16)  # loaded mid-qg

        qT_sb = singles.tile([128, HPC, QL], BF16)  # Q^T per head: [d, h, q]
        kT_sb = singles.tile([128, KV], BF16)  # K^T: [d, kv]
        v_sb = singles.tile([128, NKV, D], BF16)  # V: [kv%128, r, d]

        def rsqrt_sum(rr, ssum, n, tg):
            """rr = 1/sqrt(ssum/D + eps), all on DVE (no ACT table swap)."""
            x = work_pool.tile([128, n], F32, tag=f"rx{tg}")
            nc.vector.tensor_scalar(out=x, in0=ssum, scalar1=1.0 / D,
                                    scalar2=EPS, op0=ALU.mult, op1=ALU.add)
            yi = work_pool.tile([128, n], I32, tag=f"ry{tg}")
            nc.vector.tensor_scalar(out=yi, in0=x[:, :].bitcast(I32), scalar1=1,
                                    scalar2=None, op0=ALU.logical_shift_right)
            nc.vector.tensor_tensor(out=yi, in0=magict[:, 0:n], in1=yi,
                                    op=ALU.subtract)
            y = yi[:, :].bitcast(F32)
            t1 = work_pool.tile([128, n], F32, tag=f"rt{tg}")
            nc.vector.tensor_tensor(out=t1, in0=y, in1=y, op=ALU.mult)
            nc.vector.tensor_tensor(out=t1, in0=t1, in1=x, op=ALU.mult)
            nc.vector.tensor_scalar(out=t1, in0=t1, scalar1=-0.5, scalar2=1.5,
                                    op0=ALU.mult, op1=ALU.add)
            nc.vector.tensor_tensor(out=rr, in0=t1, in1=y, op=ALU.mult)

        def rope(ro, xn, cs, t1):
            """ro = rotate-half(xn) with cs = [cw1|sw2|cw2|sw1] slices."""
            hf = D // 2
            nc.vector.tensor_mul(ro[:, :, 0:hf], xn[:, :, 0:hf], cs[0])
            nc.vector.tensor_mul(t1, xn[:, :, hf:D], cs[1])
            nc.vector.tensor_sub(ro[:, :, 0:hf], ro[:, :, 0:hf], t1)
            nc.vector.tensor_mul(ro[:, :, hf:D], xn[:, :, hf:D], cs[2])
            nc.vector.tensor_mul(t1, xn[:, :, 0:hf], cs[3])
            nc.vector.tensor_add(ro[:, :, hf:D], ro[:, :, hf:D], t1)

        def qnorm(qe, qc, csqp, ci):
            """qe: [128, 4, 128] bf16 SBUF evac of the q projection.
            norm+rope -> 4 transposes -> qT_sb."""
            sq = work_pool.tile([128, HPC, D], BF16, tag="qsq")
            nc.vector.tensor_mul(sq, qe, qe)
            ssum = work_pool.tile([128, HPC], F32, tag="qssum")
            nc.vector.tensor_reduce(ssum, sq, axis=mybir.AxisListType.X, op=ALU.add)
            rr = work_pool.tile([128, HPC], F32, tag="qrr")
            rsqrt_sum(rr, ssum, HPC, "q")
            qn = work_pool.tile([128, HPC, D], BF16, tag="qn")
            for h in range(HPC):
                nc.vector.tensor_scalar_mul(out=qn[:, h, :], in0=qe[:, h, :],
                                            scalar1=rr[:, h:h + 1])
            hf = D // 2
            ro = work_pool.tile([128, HPC, D], BF16, tag="qro")
            t1 = work_pool.tile([128, HPC, hf], BF16, tag="qt1")
            for h in range(HPC):
                cs = [csqp[:, ci:ci + 1, s * hf:(s + 1) * hf] for s in range(4)]
                rope(ro[:, h:h + 1, :], qn[:, h:h + 1, :], cs, t1[:, 0:1, :])
            for h in range(HPC):
                tp = p1.tile([128, 128], BF16, tag="acc", name=f"tpq{qc}_{h}")
                nc.tensor.transpose(tp, ro[:, h, :], identb)
                nc.scalar.copy(out=qT_sb[:, h, qc * 128:(qc + 1) * 128], in_=tp)

        def knorm_pair(ke, r0, cskp):
            """ke: [128, 2, 2D] bf16 SBUF evac of the K|V projection pair."""
            sq = work_pool.tile([128, 2, D], BF16, tag="ksq")
            nc.vector.tensor_mul(sq, ke[:, :, 0:D], ke[:, :, 0:D])
            ssum = work_pool.tile([128, 2], F32, tag="kssum")
            nc.vector.tensor_reduce(ssum, sq, axis=mybir.AxisListType.X, op=ALU.add)
            rr = work_pool.tile([128, 2], F32, tag="krr")
            rsqrt_sum(rr, ssum, 2, "k")
            kn = work_pool.tile([128, 2, D], BF16, tag="kn")
            for c in range(2):
                nc.vector.tensor_scalar_mul(out=kn[:, c, :], in0=ke[:, c, 0:D],
                                            scalar1=rr[:, c:c + 1])
            hf = D // 2
            cs = [cskp[:, :, s * hf:(s + 1) * hf] for s in range(4)]
            ro = work_pool.tile([128, 2, D], BF16, tag="kro")
            t1 = work_pool.tile([128, 2, hf], BF16, tag="kt1")
            rope(ro, kn, cs, t1)
            for c in range(2):
                tp = p1.tile([128, 128], BF16, tag="acc", name=f"tpk{r0}_{c}")
                nc.tensor.transpose(tp, ro[:, c, :], identb)
                nc.scalar.copy(out=kT_sb[:, (r0 + c) * 128:(r0 + c + 1) * 128], in_=tp)
            nc.vector.tensor_copy(out=v_sb[:, r0:r0 + 2, :], in_=ke[:, :, D:2 * D])

        # Deferred-norm pipeline: each projection group's norm+rope+PE
        # transposes are emitted AFTER the NEXT group's matmuls so the DVE
        # chain hides behind tensor work.
        pending = []

        def flush_pending():
            while pending:
                pending.pop(0)()

        def cg_group(g):
            """Context projection: kv chunks 2g, 2g+1."""
            sl = load_ckv(g * 256)
            cskp = evac_pool.tile([128, 2, 2 * D], BF16, tag="cskp")
            nc.scalar.dma_start(out=cskp, in_=csk[:, 2 * g:2 * g + 2, :])
            pk = p1.tile([128, 2, 2 * D], F32, tag="acc", name=f"cgk{g}")
            for k in range(NHD):
                for c in range(2):
                    nc.tensor.matmul(pk[:, c, :], lhsT=sl(k, c),
                                     rhs=wkv_sb[:, k, :],
                                     start=(k == 0 and c == 0),
                                     stop=(k == NHD - 1))
            ke = evac_pool.tile([128, 2, 2 * D], BF16, tag="ke")
            nc.scalar.copy(out=ke[:], in_=pk)
            flush_pending()
            pending.append(lambda ke=ke, r0=2 * g, cskp=cskp: knorm_pair(ke, r0, cskp))

        def qg_group(g):
            """Query-row projection: kv chunks 32+2g, 33+2g (q chunks 2g,
            2g+1): shared stationary feeds both wkv and wq."""
            sl = load_ckv(CTX + g * 256)
            cskp = evac_pool.tile([128, 2, 2 * D], BF16, tag="cskp")
            nc.scalar.dma_start(out=cskp, in_=csk[:, NCTX + 2 * g:NCTX + 2 * g + 2, :])
            csqp = evac_pool.tile([128, 2, 2 * D], BF16, tag="csqp")
            nc.scalar.dma_start(out=csqp, in_=csq[:, 2 * g:2 * g + 2, :])
            pk = p1.tile([128, 2, 2 * D], F32, tag="acc", name=f"qgk{g}")
            pq = [p1.tile([128, HPC * D], F32, tag="acc", name=f"qgq{g}_{c}")
                  for c in range(2)]
            for k in range(NHD):
                for c in range(2):
                    st = sl(k, c)
                    nc.tensor.matmul(pk[:, c, :], lhsT=st, rhs=wkv_sb[:, k, :],
                                     start=(k == 0 and c == 0),
                                     stop=(k == NHD - 1))
                    nc.tensor.matmul(pq[c], lhsT=st, rhs=wq_sb[:, k, :],
                                     start=(k == 0), stop=(k == NHD - 1))
            ke = evac_pool.tile([128, 2, 2 * D], BF16, tag="ke")
            nc.scalar.copy(out=ke[:], in_=pk)
            qes = []
            for c in range(2):
                qe = evac_pool.tile([128, HPC, D], BF16, tag="qe")
                nc.scalar.copy(out=qe[:], in_=pq[c])
                qes.append(qe)
            flush_pending()

            def norm(ke=ke, qes=qes, csqp=csqp, cskp=cskp, g=g):
                knorm_pair(ke, NCTX + 2 * g, cskp)
                for c in range(2):
                    qnorm(qes[c], 2 * g + c, csqp, c)
            pending.append(norm)

        # ---- attention ----
        # finalize (den matmuls, reciprocal, normalize, AG payload write) of
        # the previous stream is deferred and emitted after the next block of
        # dense PE work has started.
        finalize_pending = []

        def flush_finalize():
            while finalize_pending:
                finalize_pending.pop(0)()

        def make_finalize(j, h, o_acc, saccs):
            def fin():
                den = p1.tile([128, 512], F32, tag="acc", name=f"den{j}_{h}")
                for s in range(4):
                    nc.tensor.matmul(den, lhsT=ones_bf,
                                     rhs=saccs[:, s * 512:(s + 1) * 512],
                                     start=(s == 0), stop=(s == 3))
                pr = work_pool.tile([128, 512], F32, tag="dps")
                nc.vector.reciprocal_approx_fast(out=pr, in_=den)
                s3 = stg_pool.tile([128, 512], BF16, tag="stg")
                nc.vector.tensor_mul(s3, o_acc, pr)
                s3v = s3[:, :].rearrange("p (c q) -> p c q", c=4)
                if j == 0:
                    nc.gpsimd.dma_start(out=ag0_ins[h][:, :, :], in_=s3v)
                    nc.gpsimd.collective_compute(
                        "AllGather", ALU.bypass, ins=[ag0_ins[h][:, :, :]],
                        outs=[ag0_outs[h][:, :, :]],
                        replica_groups=[list(range(NCORES))])
                else:
                    nc.gpsimd.dma_start(out=ag_ins[j - 1][:, :, h, :], in_=s3v)
                    if h == HPC - 1:
                        nc.gpsimd.collective_compute(
                            "AllGather", ALU.bypass,
                            ins=[ag_ins[j - 1][:, :, :, :]],
                            outs=[ag_outs[j - 1][:, :, :, :]],
                            replica_groups=[list(range(NCORES))])
            return fin

        def attn_stream(j, h, split_at=None):
            """One (j,h) attention stream over all its kv chunks.
            split_at: optional chunk index at which to flush the projection
            norm pipeline (used once, to hide the last qg group's norm)."""
            r_list = list(range(NCTX + 4 + 4 * j))
            nq = len(r_list) // 4
            qTj = qT_sb[:, h, j * 512:(j + 1) * 512]
            o_acc = p1.tile([128, 512], F32, tag="acc", name=f"o{j}_{h}")
            saccs = saccs_pool.tile([128, 4 * 512], BF16, tag="sacc")
            for qd in range(nq):
                if split_at is not None and qd * 4 == split_at:
                    flush_pending()
                ptw = pt_pool.tile([128, 4 * 512], BF16, tag="ptw")
                sts = []
                for pi in range(2):
                    rA = r_list[qd * 4 + 2 * pi]
                    rB = r_list[qd * 4 + 2 * pi + 1]
                    st = p2.tile([128, 1024], F32, tag="st")
                    for idx, r in ((0, rA), (1, rB)):
                        # causal mask for diagonal chunks: pre-bias the PSUM
                        # with the mask via a cheap identity matmul, then let
                        # the score matmul accumulate onto it (keeps the DVE
                        # off the exp critical path).
                        i = r - NCTX - 4 * j
                        diag = 0 <= i <= 3
                        if diag:
                            nc.tensor.matmul(
                                st[:, idx * 512:(idx + 1) * 512], lhsT=identb,
                                rhs=msk_sb[:, 384 - 128 * i:896 - 128 * i],
                                start=True, stop=False)
                        nc.tensor.matmul(st[:, idx * 512:(idx + 1) * 512],
                                         lhsT=kT_sb[:, r * 128:(r + 1) * 128],
                                         rhs=qTj, start=not diag, stop=True)
                    sts.append(st)
                for pi in range(2):
                    nc.scalar.activation(out=ptw[:, pi * 1024:(pi + 1) * 1024],
                                         in_=sts[pi][:],
                                         func=mybir.ActivationFunctionType.Exp,
                                         bias=zbias, scale=SCALE)
                for slot in range(4):
                    r = r_list[qd * 4 + slot]
                    nc.tensor.matmul(
                        o_acc, lhsT=v_sb[:, r, :],
                        rhs=ptw[:, slot * 512:(slot + 1) * 512],
                        start=(qd == 0 and slot == 0),
                        stop=(qd == nq - 1 and slot == 3))
                if qd == 0:
                    flush_finalize()
                    nc.vector.tensor_copy(out=saccs, in_=ptw)
                else:
                    nc.vector.tensor_add(saccs, saccs, ptw)
            finalize_pending.append(make_finalize(j, h, o_acc, saccs))

        def oproj_block(jq):
            """o_proj for q tile jq (jq >= 1) from the AllGathered attn^T.
            Tiles load per (qc, 4-core half) as 1KB-contiguous segments."""
            ats = {}

            def at_load(hv):
                qc2, lo = hv // 2, (hv % 2) * 4
                at = at_pool.tile([128, 4, HPC, 128], BF16, tag="at")
                nc.sync.dma_start(
                    out=at,
                    in_=ag_outs[jq - 1][lo * 128:(lo + 4) * 128, qc2, :, :]
                    .rearrange("(c p) h q -> p c h q", p=128))
                ats[hv] = at

            at_load(0)
            at_load(1)
            for qc2 in range(4):
                qc = 4 * jq + qc2
                po = p1.tile([128, HPC * D], F32, tag="acc", name=f"po{qc}")
                for half in range(2):
                    at = ats.pop(qc2 * 2 + half)
                    for c4 in range(4):
                        for h2 in range(HPC):
                            gg = (half * 4 + c4) * HPC + h2
                            nc.tensor.matmul(po, lhsT=at[:, c4, h2, :],
                                             rhs=wo_sb[:, gg, :],
                                             start=(gg == 0), stop=(gg == H - 1))
                    if qc2 == 0 and half == 0:
                        flush_finalize()
                    if qc2 * 2 + half + 2 < 8:
                        at_load(qc2 * 2 + half + 2)
                ot = stg_pool.tile([128, HPC * D], F32, tag="ot")
                nc.scalar.copy(out=ot, in_=po)
                nc.sync.dma_start(out=out_ext[qc * 128:(qc + 1) * 128, :], in_=ot)

        def oproj_block0():
            """o_proj for q tile 0, head-major: the h2<3 contraction chunks
            run while AG(0, h=3) is still in flight, so only the last-quarter
            matmuls trail the final collective."""
            pos = [p1.tile([128, HPC * D], F32, tag="acc", name=f"po0_{qc2}")
                   for qc2 in range(4)]
            ats = {}

            def at0_load(h2, qc2):
                at = at_pool.tile([128, NCORES, 128], BF16, tag="at")
                nc.sync.dma_start(
                    out=at,
                    in_=ag0_outs[h2][:, qc2, :].rearrange("(c p) q -> p c q",
                                                          p=128))
                ats[(h2, qc2)] = at

            at0_load(0, 0)
            at0_load(0, 1)
            order = [(h2, qc2) for h2 in range(HPC) for qc2 in range(4)]
            for oi, (h2, qc2) in enumerate(order):
                at = ats.pop((h2, qc2))
                po = pos[qc2]
                for cc in range(NCORES):
                    nc.tensor.matmul(po, lhsT=at[:, cc, :],
                                     rhs=wo_sb[:, cc * HPC + h2, :],
                                     start=(h2 == 0 and cc == 0),
                                     stop=(h2 == HPC - 1 and cc == NCORES - 1))
                if oi + 2 < len(order):
                    at0_load(*order[oi + 2])
            for qc2 in range(4):
                ot = stg_pool.tile([128, HPC * D], F32, tag="ot")
                nc.scalar.copy(out=ot, in_=pos[qc2])
                nc.sync.dma_start(out=out_ext[qc2 * 128:(qc2 + 1) * 128, :],
                                  in_=ot)

        # ---- main sequence ----
        cg_group(0)
        nc.sync.dma_start(out=msk_sb[:], in_=mskd[:, :])
        for g in range(1, 6):
            cg_group(g)
        nc.scalar.dma_start(out=wq_sb[:], in_=wq3[:, :, :])
        for g in range(6, 16):
            cg_group(g)
        # qg group 0 (q chunks 0,1 / kv 32,33) runs LAST so its deferred norm
        # can flush inside the first attention stream (which needs q chunks
        # 12-15 and all kv, but touches kv>=32 only from its 9th quad on).
        qg_group(1)
        qg_group(2)
        nc.sync.dma_start(out=wo_sb[:], in_=wo3[:, :, :])
        for g in (3, 4, 5, 6, 7, 0):
            qg_group(g)

        attn_stream(3, 0, split_at=32)
        attn_stream(3, 1)
        attn_stream(3, 2)
        attn_stream(3, 3)
        for h in range(HPC):
            attn_stream(2, h)
        oproj_block(3)
        for h in range(HPC):
            attn_stream(1, h)
        oproj_block(2)
        for h in range(HPC):
            attn_stream(0, h)
        flush_finalize()  # fires AG(0,3) immediately; oproj(1) hides it
        oproj_block(1)
        oproj_block0()

    nc.compile()
    return nc


def _tile_hid(a):
    """[HID, C] -> [128, NHD, C] with (p, k, c) = a[k*128+p, c]."""
    return np.ascontiguousarray(
        a.reshape(NHD, 128, a.shape[1]).transpose(1, 0, 2)).astype(bfloat16)


def _host_prep(context, query, w_qkv, w_o, q_norm_w, k_norm_w):
    context = np.asarray(context, dtype=np.float32)
    query = np.asarray(query, dtype=np.float32)
    w_qkv = np.asarray(w_qkv, dtype=np.float32)
    w_o = np.asarray(w_o, dtype=np.float32)
    q_norm_w = np.asarray(q_norm_w, dtype=np.float32)
    k_norm_w = np.asarray(k_norm_w, dtype=np.float32)

    ck = np.concatenate([context, query], axis=0)  # [KV, HID]
    ckT = np.ascontiguousarray(ck.T)  # [HID, KV] f32
    # [g][p][k][c] blocks of 256 kv columns (one projection group each)
    ckT4 = np.ascontiguousarray(
        ckT.reshape(NHD, 128, KV // 256, 256).transpose(2, 1, 0, 3)
    ).astype(bfloat16)

    wq = w_qkv[:, :H * D]
    wk = w_qkv[:, H * D:H * D + KVH * D]
    wv = w_qkv[:, H * D + KVH * D:]

    half = D // 2
    inv_freq = (1.0 / (THETA ** (np.arange(0, half, dtype=np.float32) / half))
                ).astype(np.float32)
    pos = np.arange(KV, dtype=np.float32)
    freqs = pos[:, None] * inv_freq[None, :]
    cosf, sinf = np.cos(freqs), np.sin(freqs)  # [KV, 64]

    def cs4(w):
        # [cos*w1 | sin*w2 | cos*w2 | sin*w1], w1 = w[:64], w2 = w[64:]
        return np.concatenate([cosf * w[None, :half], sinf * w[None, half:],
                               cosf * w[None, half:], sinf * w[None, :half]],
                              axis=1).astype(np.float32)  # [KV, 2D]

    cs4q = cs4(q_norm_w)[CTX:]  # [QL, 2D]
    cs4k = cs4(k_norm_w)  # [KV, 2D]
    csq_t = np.ascontiguousarray(
        cs4q.reshape(NQC, 128, 2 * D).transpose(1, 0, 2)).astype(bfloat16)
    csk_t = np.ascontiguousarray(
        cs4k.reshape(NKV, 128, 2 * D).transpose(1, 0, 2)).astype(bfloat16)

    p = np.arange(128)[:, None]
    t = np.arange(896)[None, :]
    msk = np.where(p <= t - 384, 0.0, MASKVAL).astype(bfloat16)  # [128, 896]

    in_maps = []
    for c in range(NCORES):
        in_maps.append({
            "ckT4": ckT4,
            "wq3": _tile_hid(wq[:, c * HPC * D:(c + 1) * HPC * D]),
            "wkv3": _tile_hid(np.concatenate(
                [wk[:, c * D:(c + 1) * D], wv[:, c * D:(c + 1) * D]], axis=1)),
            "wo3": _tile_hid(w_o[:, c * HPC * D:(c + 1) * HPC * D]),
            "csq": csq_t,
            "csk": csk_t,
            "msk": msk,
        })
    return in_maps


def kernel(context, query, w_qkv, w_o, q_norm_w, k_norm_w, **kw):
    if "nc" not in _STATE:
        _STATE["nc"] = _build()
    nc = _STATE["nc"]
    in_maps = _host_prep(context, query, w_qkv, w_o, q_norm_w, k_norm_w)
    res = run_bass_kernel_spmd(nc, in_maps, list(range(NCORES)), **kw)
    out = np.concatenate([np.asarray(res.results[c]["out"]) for c in range(NCORES)],
                         axis=1)
    if kw:
        return out.astype(np.float32), res
    return out.astype(np.float32)


# revision 28
# speedup vs baseline: 1.3148x; 1.0056x over previous
"""DFlash Qwen3 cross-attention on 8 TRN2 NeuronCores (v3).

Sharding: tensor-parallel over heads. Core c owns KV head c (KVH=8) and the
4 query heads 4c..4c+3 of its GQA group. Each core computes its heads'
QKV projections, per-head RMSNorm + RoPE, causal attention; per q-tile j the
normalized attention outputs (transposed, [4*D, 512]) are AllGathered so
every core holds attn^T for all 32 heads; each core computes a 512-column
slice of o_proj and the host concatenates the 8 slices.

v3 structure (vs v2, driven by the NTFF profile of v2):
- phase order: context proj (16 groups) -> query proj (groups 1..7 then 0)
  -> attention j=3,2,1,0 -> o_proj j' trailing one attention block. The last
  attention block is the cheapest (j=0), and o_proj(1) hides AG(0) latency.
- bubble-free attention quads: both score-pair matmuls emitted before the
  exps, AV matmuls after, so the PE never waits a full exp latency mid-quad.
- stream finalize (denominator matmuls, reciprocal, normalize, AllGather
  payload write) deferred into the NEXT stream after its first quad: kills
  the ~2.8us PE stall + HAM re-throttle at every stream boundary.
- per-j single AllGather with a [128, 4(qc), 4(h), 128] payload so o_proj
  reads back [128, 8c, 4h, 128] tiles as 1KB-contiguous segments, prefetched
  double-buffered on the idle sync DMA queue.
- RMSNorm rsqrt fully on DVE (quake bit-trick + 1 Newton step): the ACT
  engine runs exp only -- no ACT table swaps (v2 paid 27 x 1.3us mid-attention).
- wkv weight DMA split in 4 + non-critical loads (wq/wo/msk) deferred past
  the startup barrier window so the first matmul issues ~40us earlier.
"""

from contextlib import ExitStack

import numpy as np
from ml_dtypes import bfloat16

import concourse.bass as bass
import concourse.bass_isa as bass_isa
import concourse.mybir as mybir
import concourse.tile as tile
from concourse import bacc
from concourse.bass_utils import run_bass_kernel_spmd
from concourse.masks import make_identity

H = 32
KVH = 8
D = 128
HID = 4096
CTX = 4096
QL = 2048
KV = CTX + QL  # 6144
NCORES = 8
HPC = H // NCORES  # 4 q heads per core
THETA = 1000000.0
EPS = 1e-6
SCALE = float(D) ** -0.5

NHD = HID // 128  # 32 contraction chunks
NKV = KV // 128  # 48 kv chunks
NQC = QL // 128  # 16 q row chunks
NQJ = QL // 512  # 4 q column tiles for attention
NCTX = CTX // 128  # 32 context kv chunks
MASKVAL = -1e6
MAGIC = 0x5F3759DF  # fast inverse sqrt seed
SCHR_A = (128.0 / np.log(2.0)) * SCALE  # bf16 Schraudolph exp slope
SCHR_B = 127.0 * 128.0 - 7.5  # bf16 Schraudolph exp bias (tuned offline)

F32 = mybir.dt.float32
BF16 = mybir.dt.bfloat16
I32 = mybir.dt.int32
ALU = mybir.AluOpType

_STATE = {}


def _build():
    nc = bacc.Bacc()

    # ck^T in 256-kv-column blocks, [g][p][k][c] so one group load is an
    # 8KB-contiguous segment per partition (cheap descriptors, full HBM bw).
    ckT4 = nc.declare_dram_parameter("ckT4", [KV // 256, 128, NHD, 256], BF16,
                                     isOutput=False)
    wq3 = nc.declare_dram_parameter("wq3", [128, NHD, HPC * D], BF16, isOutput=False)
    wkv3 = nc.declare_dram_parameter("wkv3", [128, NHD, 2 * D], BF16, isOutput=False)
    wo3 = nc.declare_dram_parameter("wo3", [128, NHD, HPC * D], BF16, isOutput=False)
    csq = nc.declare_dram_parameter("csq", [128, NQC, 2 * D], BF16, isOutput=False)
    csk = nc.declare_dram_parameter("csk", [128, NKV, 2 * D], BF16, isOutput=False)
    mskd = nc.declare_dram_parameter("msk", [128, 896], BF16, isOutput=False)
    out_ext = nc.declare_dram_parameter("out", [QL, HPC * D], F32, isOutput=True)

    warm_in = nc.dram_tensor("warm_in", [128, 8], BF16)
    warm_out = nc.dram_tensor("warm_out", [NCORES * 128, 8], BF16, addr_space="Shared")
    # AllGather payload per j: [128 d-part, 4 qc, 4 h, 128 q] so the o_proj
    # readback for one qc is 1KB-contiguous per core block. j=0 (the LAST
    # attention block) gathers per head instead so only a 1MB collective and
    # the h=3 quarter of o_proj(0) remain on the critical tail.
    ag_ins = [nc.dram_tensor(f"ag_in{j}", [128, 4, HPC, 128], BF16)
              for j in range(1, NQJ)]
    ag_outs = [nc.dram_tensor(f"ag_out{j}", [NCORES * 128, 4, HPC, 128], BF16,
                              addr_space="Shared") for j in range(1, NQJ)]
    ag0_ins = [nc.dram_tensor(f"ag0_in{h}", [128, 4, 128], BF16)
               for h in range(HPC)]
    ag0_outs = [nc.dram_tensor(f"ag0_out{h}", [NCORES * 128, 4, 128], BF16,
                               addr_space="Shared") for h in range(HPC)]

    with tile.TileContext(nc) as tc, ExitStack() as ctx:
        singles = ctx.enter_context(tc.tile_pool(name="singles", bufs=1))
        ckv_pool = ctx.enter_context(tc.tile_pool(name="ckv", bufs=3))
        evac_pool = ctx.enter_context(tc.tile_pool(name="evac", bufs=3))
        work_pool = ctx.enter_context(tc.tile_pool(name="work", bufs=2))
        pt_pool = ctx.enter_context(tc.tile_pool(name="ptp", bufs=3))
        saccs_pool = ctx.enter_context(tc.tile_pool(name="sac", bufs=2))
        stg_pool = ctx.enter_context(tc.tile_pool(name="stg", bufs=2))
        at_pool = ctx.enter_context(tc.tile_pool(name="atp", bufs=2))
        # PSUM: 4 x 1-bank accumulators + 2 x 2-bank score tiles = 8 banks
        p1 = ctx.enter_context(tc.tile_pool(name="p1", bufs=4, space="PSUM"))
        p2 = ctx.enter_context(tc.tile_pool(name="p2", bufs=2, space="PSUM"))

        # ---- critical-path DMAs first: wkv (split x4) + first ck chunks ----
        wkv_sb = singles.tile([128, NHD, 2 * D], BF16)
        for q4 in range(4):
            nc.scalar.dma_start(out=wkv_sb[:, q4 * 8:(q4 + 1) * 8, :],
                                in_=wkv3[:, q4 * 8:(q4 + 1) * 8, :])

        def load_ckv(c0):
            """Stream ck^T block c0//256 in two k-halves (sync queue, so
            the warmup collective trigger on gpsimd can't block them)."""
            g2 = c0 // 256
            ca = ckv_pool.tile([128, NHD // 2, 256], BF16, tag="ckv")
            nc.sync.dma_start(out=ca, in_=ckT4[g2, :, 0:NHD // 2, :])
            cb = ckv_pool.tile([128, NHD // 2, 256], BF16, tag="ckv")
            nc.sync.dma_start(out=cb, in_=ckT4[g2, :, NHD // 2:NHD, :])

            def sl(k, c):
                t = ca if k < NHD // 2 else cb
                return t[:, k % (NHD // 2), c * 128:(c + 1) * 128]
            return sl

        # ---- warmup collective: pay the first-collective handshake now ----
        wtile = singles.tile([128, 8], BF16)
        nc.vector.memset(wtile, 0.0)
        nc.gpsimd.dma_start(out=warm_in[:, :], in_=wtile)
        nc.gpsimd.collective_compute(
            "AllGather", ALU.bypass, ins=[warm_in[:, :]], outs=[warm_out[:, :]],
            replica_groups=[list(range(NCORES))])

        # ---- constants ----
        identb = singles.tile([128, 128], BF16)
        make_identity(nc, identb)
        ones_bf = singles.tile([128, 128], BF16)
        nc.vector.memset(ones_bf, 1.0)
        zbias = singles.tile([128, 1], F32)
        nc.vector.memset(zbias, 0.0)
        magict = singles.tile([128, 4], I32)
        nc.vector.memset(magict, MAGIC)

        wq_sb = singles.tile([128, NHD, HPC * D], BF16)  # loaded mid-cg
        # compact causal mask: msk_sb[p, 384-128i + q] = 0 if 128i+p <= q
        # else MASKVAL, so chunk i uses the slice [384-128i : 896-128i].
        msk_sb = singles.tile([128, 896], BF16)  # loaded after cg0
        wo_sb = singles.tile([128, NHD, HPC * D], BF16)  # loaded mid-qg

        qT_sb = singles.tile([128, HPC, QL], BF16)  # Q^T per head: [d, h, q]
        kT_sb = singles.tile([128, KV], BF16)  # K^T: [d, kv]
        v_sb = singles.tile([128, NKV, D], BF16)  # V: [kv%128, r, d]

        def rsqrt_sum(rr, ssum, n, tg):
            """rr = 1/sqrt(ssum/D + eps), all on DVE (no ACT table swap)."""
            x = work_pool.tile([128, n], F32, tag=f"rx{tg}")
            nc.vector.tensor_scalar(out=x, in0=ssum, scalar1=1.0 / D,
                                    scalar2=EPS, op0=ALU.mult, op1=ALU.add)
            yi = work_pool.tile([128, n], I32, tag=f"ry{tg}")
            nc.vector.tensor_scalar(out=yi, in0=x[:, :].bitcast(I32), scalar1=1,
                                    scalar2=None, op0=ALU.logical_shift_right)
            nc.vector.tensor_tensor(out=yi, in0=magict[:, 0:n], in1=yi,
                                    op=ALU.subtract)
            y = yi[:, :].bitcast(F32)
            t1 = work_pool.tile([128, n], F32, tag=f"rt{tg}")
            nc.vector.tensor_tensor(out=t1, in0=y, in1=y, op=ALU.mult)
            nc.vector.tensor_tensor(out=t1, in0=t1, in1=x, op=ALU.mult)
            nc.vector.tensor_scalar(out=t1, in0=t1, scalar1=-0.5, scalar2=1.5,
                                    op0=ALU.mult, op1=ALU.add)
            nc.vector.tensor_tensor(out=rr, in0=t1, in1=y, op=ALU.mult)

        def rope(ro, xn, cs, t1):
            """ro = rotate-half(xn) with cs = [cw1|sw2|cw2|sw1] slices."""
            hf = D // 2
            nc.vector.tensor_mul(ro[:, :, 0:hf], xn[:, :, 0:hf], cs[0])
            nc.vector.tensor_mul(t1, xn[:, :, hf:D], cs[1])
            nc.vector.tensor_sub(ro[:, :, 0:hf], ro[:, :, 0:hf], t1)
            nc.vector.tensor_mul(ro[:, :, hf:D], xn[:, :, hf:D], cs[2])
            nc.vector.tensor_mul(t1, xn[:, :, 0:hf], cs[3])
            nc.vector.tensor_add(ro[:, :, hf:D], ro[:, :, hf:D], t1)

        def qnorm(qe, qc, csqp, ci):
            """qe: [128, 4, 128] bf16 SBUF evac of the q projection.
            norm+rope -> 4 transposes -> qT_sb."""
            sq = work_pool.tile([128, HPC, D], BF16, tag="qsq")
            nc.vector.tensor_mul(sq, qe, qe)
            ssum = work_pool.tile([128, HPC], F32, tag="qssum")
            nc.vector.tensor_reduce(ssum, sq, axis=mybir.AxisListType.X, op=ALU.add)
            rr = work_pool.tile([128, HPC], F32, tag="qrr")
            rsqrt_sum(rr, ssum, HPC, "q")
            qn = work_pool.tile([128, HPC, D], BF16, tag="qn")
            for h in range(HPC):
                nc.vector.tensor_scalar_mul(out=qn[:, h, :], in0=qe[:, h, :],
                                            scalar1=rr[:, h:h + 1])
            hf = D // 2
            ro = work_pool.tile([128, HPC, D], BF16, tag="qro")
            t1 = work_pool.tile([128, HPC, hf], BF16, tag="qt1")
            for h in range(HPC):
                cs = [csqp[:, ci:ci + 1, s * hf:(s + 1) * hf] for s in range(4)]
                rope(ro[:, h:h + 1, :], qn[:, h:h + 1, :], cs, t1[:, 0:1, :])
            for h in range(HPC):
                tp = p1.tile([128, 128], BF16, tag="acc", name=f"tpq{qc}_{h}")
                nc.tensor.transpose(tp, ro[:, h, :], identb)
                nc.scalar.copy(out=qT_sb[:, h, qc * 128:(qc + 1) * 128], in_=tp)

        def knorm_pair(ke, r0, cskp):
            """ke: [128, 2, 2D] bf16 SBUF evac of the K|V projection pair."""
            sq = work_pool.tile([128, 2, D], BF16, tag="ksq")
            nc.vector.tensor_mul(sq, ke[:, :, 0:D], ke[:, :, 0:D])
            ssum = work_pool.tile([128, 2], F32, tag="kssum")
            nc.vector.tensor_reduce(ssum, sq, axis=mybir.AxisListType.X, op=ALU.add)
            rr = work_pool.tile([128, 2], F32, tag="krr")
            rsqrt_sum(rr, ssum, 2, "k")
            kn = work_pool.tile([128, 2, D], BF16, tag="kn")
            for c in range(2):
                nc.vector.tensor_scalar_mul(out=kn[:, c, :], in0=ke[:, c, 0:D],
                                            scalar1=rr[:, c:c + 1])
            hf = D // 2
            cs = [cskp[:, :, s * hf:(s + 1) * hf] for s in range(4)]
            ro = work_pool.tile([128, 2, D], BF16, tag="kro")
            t1 = work_pool.tile([128, 2, hf], BF16, tag="kt1")
            rope(ro, kn, cs, t1)
            for c in range(2):
                tp = p1.tile([128, 128], BF16, tag="acc", name=f"tpk{r0}_{c}")
                nc.tensor.transpose(tp, ro[:, c, :], identb)
                nc.scalar.copy(out=kT_sb[:, (r0 + c) * 128:(r0 + c + 1) * 128], in_=tp)
            nc.vector.tensor_copy(out=v_sb[:, r0:r0 + 2, :], in_=ke[:, :, D:2 * D])

        # Deferred-norm pipeline: each projection group's norm+rope+PE
        # transposes are emitted AFTER the NEXT group's matmuls so the DVE
        # chain hides behind tensor work.
        pending = []

        def flush_pending():
            while pending:
                pending.pop(0)()

        def cg_group(g):
            """Context projection: kv chunks 2g, 2g+1."""
            sl = load_ckv(g * 256)
            cskp = evac_pool.tile([128, 2, 2 * D], BF16, tag="cskp")
            nc.scalar.dma_start(out=cskp, in_=csk[:, 2 * g:2 * g + 2, :])
            pk = p1.tile([128, 2, 2 * D], F32, tag="acc", name=f"cgk{g}")
            for k in range(NHD):
                for c in range(2):
                    nc.tensor.matmul(pk[:, c, :], lhsT=sl(k, c),
                                     rhs=wkv_sb[:, k, :],
                                     start=(k == 0 and c == 0),
                                     stop=(k == NHD - 1))
            ke = evac_pool.tile([128, 2, 2 * D], BF16, tag="ke")
            nc.scalar.copy(out=ke[:], in_=pk)
            flush_pending()
            pending.append(lambda ke=ke, r0=2 * g, cskp=cskp: knorm_pair(ke, r0, cskp))

        def qg_group(g):
            """Query-row projection: kv chunks 32+2g, 33+2g (q chunks 2g,
            2g+1): shared stationary feeds both wkv and wq."""
            sl = load_ckv(CTX + g * 256)
            cskp = evac_pool.tile([128, 2, 2 * D], BF16, tag="cskp")
            nc.scalar.dma_start(out=cskp, in_=csk[:, NCTX + 2 * g:NCTX + 2 * g + 2, :])
            csqp = evac_pool.tile([128, 2, 2 * D], BF16, tag="csqp")
            nc.scalar.dma_start(out=csqp, in_=csq[:, 2 * g:2 * g + 2, :])
            pk = p1.tile([128, 2, 2 * D], F32, tag="acc", name=f"qgk{g}")
            pq = [p1.tile([128, HPC * D], F32, tag="acc", name=f"qgq{g}_{c}")
                  for c in range(2)]
            for k in range(NHD):
                for c in range(2):
                    st = sl(k, c)
                    nc.tensor.matmul(pk[:, c, :], lhsT=st, rhs=wkv_sb[:, k, :],
                                     start=(k == 0 and c == 0),
                                     stop=(k == NHD - 1))
                    nc.tensor.matmul(pq[c], lhsT=st, rhs=wq_sb[:, k, :],
                                     start=(k == 0), stop=(k == NHD - 1))
            ke = evac_pool.tile([128, 2, 2 * D], BF16, tag="ke")
            nc.scalar.copy(out=ke[:], in_=pk)
            qes = []
            for c in range(2):
                qe = evac_pool.tile([128, HPC, D], BF16, tag="qe")
                nc.scalar.copy(out=qe[:], in_=pq[c])
                qes.append(qe)
            flush_pending()

            def norm(ke=ke, qes=qes, csqp=csqp, cskp=cskp, g=g):
                knorm_pair(ke, NCTX + 2 * g, cskp)
                for c in range(2):
                    qnorm(qes[c], 2 * g + c, csqp, c)
            pending.append(norm)

        # ---- attention ----
        # finalize (den matmuls, reciprocal, normalize, AG payload write) of
        # the previous stream is deferred and emitted after the next block of
        # dense PE work has started.
        finalize_pending = []

        def flush_finalize():
            while finalize_pending:
                finalize_pending.pop(0)()

        def make_finalize(j, h, o_acc, saccs):
            def fin():
                den = p1.tile([128, 512], F32, tag="acc", name=f"den{j}_{h}")
                for s in range(4):
                    nc.tensor.matmul(den, lhsT=ones_bf,
                                     rhs=saccs[:, s * 512:(s + 1) * 512],
                                     start=(s == 0), stop=(s == 3))
                pr = work_pool.tile([128, 512], F32, tag="dps")
                nc.vector.reciprocal_approx_fast(out=pr, in_=den)
                s3 = stg_pool.tile([128, 512], BF16, tag="stg")
                nc.vector.tensor_mul(s3, o_acc, pr)
                s3v = s3[:, :].rearrange("p (c q) -> p c q", c=4)
                if j == 0:
                    nc.gpsimd.dma_start(out=ag0_ins[h][:, :, :], in_=s3v)
                    nc.gpsimd.collective_compute(
                        "AllGather", ALU.bypass, ins=[ag0_ins[h][:, :, :]],
                        outs=[ag0_outs[h][:, :, :]],
                        replica_groups=[list(range(NCORES))])
                else:
                    nc.gpsimd.dma_start(out=ag_ins[j - 1][:, :, h, :], in_=s3v)
                    if h == HPC - 1:
                        nc.gpsimd.collective_compute(
                            "AllGather", ALU.bypass,
                            ins=[ag_ins[j - 1][:, :, :, :]],
                            outs=[ag_outs[j - 1][:, :, :, :]],
                            replica_groups=[list(range(NCORES))])
            return fin

        def attn_stream(j, h, split_at=None):
            """One (j,h) attention stream over all its kv chunks.
            split_at: optional chunk index at which to flush the projection
            norm pipeline (used once, to hide the last qg group's norm)."""
            r_list = list(range(NCTX + 4 + 4 * j))
            nq = len(r_list) // 4
            qTj = qT_sb[:, h, j * 512:(j + 1) * 512]
            o_acc = p1.tile([128, 512], F32, tag="acc", name=f"o{j}_{h}")
            saccs = saccs_pool.tile([128, 4 * 512], BF16, tag="sacc")
            for qd in range(nq):
                if split_at is not None and qd * 4 == split_at:
                    flush_pending()
                ptw = pt_pool.tile([128, 4 * 512], BF16, tag="ptw")
                sts = []
                for pi in range(2):
                    rA = r_list[qd * 4 + 2 * pi]
                    rB = r_list[qd * 4 + 2 * pi + 1]
                    st = p2.tile([128, 1024], F32, tag="st")
                    for idx, r in ((0, rA), (1, rB)):
                        # causal mask for diagonal chunks: pre-bias the PSUM
                        # with the mask via a cheap identity matmul, then let
                        # the score matmul accumulate onto it (keeps the DVE
                        # off the exp critical path).
                        i = r - NCTX - 4 * j
                        diag = 0 <= i <= 3
                        if diag:
                            nc.tensor.matmul(
                                st[:, idx * 512:(idx + 1) * 512], lhsT=identb,
                                rhs=msk_sb[:, 384 - 128 * i:896 - 128 * i],
                                start=True, stop=False)
                        nc.tensor.matmul(st[:, idx * 512:(idx + 1) * 512],
                                         lhsT=kT_sb[:, r * 128:(r + 1) * 128],
                                         rhs=qTj, start=not diag, stop=True)
                    sts.append(st)
                for pi in range(2):
                    # Every 4th score pair (never a masked/diagonal one) takes
                    # the bf16-Schraudolph path on DVE instead of ACT exp:
                    # bf16_bits(exp(x*SCALE)) ~= int16(128/ln2*SCALE*x + B).
                    # The constant bias cancels in the softmax normalization;
                    # the ~1.8% rms sawtooth on 1/4 of the weights is well
                    # inside the error budget. Keeps the ACT engine from
                    # pacing the attention phase.
                    rA = r_list[qd * 4 + 2 * pi]
                    rB = r_list[qd * 4 + 2 * pi + 1]
                    if pi == 0 and qd % 2 == 0 and rB < NCTX:
                        nc.vector.tensor_scalar(
                            out=ptw[:, 0:1024].bitcast(mybir.dt.int16),
                            in0=sts[pi][:], scalar1=SCHR_A, scalar2=SCHR_B,
                            op0=ALU.mult, op1=ALU.add)
                    else:
                        nc.scalar.activation(
                            out=ptw[:, pi * 1024:(pi + 1) * 1024],
                            in_=sts[pi][:],
                            func=mybir.ActivationFunctionType.Exp,
                            bias=zbias, scale=SCALE)
                for slot in range(4):
                    r = r_list[qd * 4 + slot]
                    nc.tensor.matmul(
                        o_acc, lhsT=v_sb[:, r, :],
                        rhs=ptw[:, slot * 512:(slot + 1) * 512],
                        start=(qd == 0 and slot == 0),
                        stop=(qd == nq - 1 and slot == 3))
                if qd == 0:
                    flush_finalize()
                    nc.vector.tensor_copy(out=saccs, in_=ptw)
                else:
                    nc.vector.tensor_add(saccs, saccs, ptw)
            finalize_pending.append(make_finalize(j, h, o_acc, saccs))

        def at_load(jq, ats, hv):
            qc2, lo = hv // 2, (hv % 2) * 4
            at = at_pool.tile([128, 4, HPC, 128], BF16, tag="at")
            nc.sync.dma_start(
                out=at,
                in_=ag_outs[jq - 1][lo * 128:(lo + 4) * 128, qc2, :, :]
                .rearrange("(c p) h q -> p c h q", p=128))
            ats[hv] = at

        def oproj_prefetch(jq):
            """Issue the first two attn^T tile loads for oproj_block(jq)
            early, so its first matmuls don't expose the DMA latency."""
            ats = {}
            at_load(jq, ats, 0)
            at_load(jq, ats, 1)
            return ats

        def oproj_block(jq, ats):
            """o_proj for q tile jq (jq >= 1) from the AllGathered attn^T.
            Tiles load per (qc, 4-core half) as 1KB-contiguous segments."""
            for qc2 in range(4):
                qc = 4 * jq + qc2
                po = p1.tile([128, HPC * D], F32, tag="acc", name=f"po{qc}")
                for half in range(2):
                    at = ats.pop(qc2 * 2 + half)
                    for c4 in range(4):
                        for h2 in range(HPC):
                            gg = (half * 4 + c4) * HPC + h2
                            nc.tensor.matmul(po, lhsT=at[:, c4, h2, :],
                                             rhs=wo_sb[:, gg, :],
                                             start=(gg == 0), stop=(gg == H - 1))
                    if qc2 == 0 and half == 0:
                        flush_finalize()
                    if qc2 * 2 + half + 2 < 8:
                        at_load(jq, ats, qc2 * 2 + half + 2)
                ot = stg_pool.tile([128, HPC * D], F32, tag="ot")
                nc.scalar.copy(out=ot, in_=po)
                nc.sync.dma_start(out=out_ext[qc * 128:(qc + 1) * 128, :], in_=ot)

        def oproj_block0():
            """o_proj for q tile 0, head-major: the h2<3 contraction chunks
            run while AG(0, h=3) is still in flight, so only the last-quarter
            matmuls trail the final collective."""
            pos = [p1.tile([128, HPC * D], F32, tag="acc", name=f"po0_{qc2}")
                   for qc2 in range(4)]
            ats = {}

            def at0_load(h2, qc2):
                at = at_pool.tile([128, NCORES, 128], BF16, tag="at")
                nc.sync.dma_start(
                    out=at,
                    in_=ag0_outs[h2][:, qc2, :].rearrange("(c p) q -> p c q",
                                                          p=128))
                ats[(h2, qc2)] = at

            at0_load(0, 0)
            at0_load(0, 1)
            order = [(h2, qc2) for h2 in range(HPC) for qc2 in range(4)]
            for oi, (h2, qc2) in enumerate(order):
                at = ats.pop((h2, qc2))
                po = pos[qc2]
                for cc in range(NCORES):
                    nc.tensor.matmul(po, lhsT=at[:, cc, :],
                                     rhs=wo_sb[:, cc * HPC + h2, :],
                                     start=(h2 == 0 and cc == 0),
                                     stop=(h2 == HPC - 1 and cc == NCORES - 1))
                if oi + 2 < len(order):
                    at0_load(*order[oi + 2])
            for qc2 in range(4):
                ot = stg_pool.tile([128, HPC * D], F32, tag="ot")
                nc.scalar.copy(out=ot, in_=pos[qc2])
                nc.sync.dma_start(out=out_ext[qc2 * 128:(qc2 + 1) * 128, :],
                                  in_=ot)

        # ---- main sequence ----
        cg_group(0)
        nc.sync.dma_start(out=msk_sb[:], in_=mskd[:, :])
        for g in range(1, 6):
            cg_group(g)
        nc.scalar.dma_start(out=wq_sb[:], in_=wq3[:, :, :])
        for g in range(6, 16):
            cg_group(g)
        # qg group 0 (q chunks 0,1 / kv 32,33) runs LAST so its deferred norm
        # can flush inside the first attention stream (which needs q chunks
        # 12-15 and all kv, but touches kv>=32 only from its 9th quad on).
        qg_group(1)
        qg_group(2)
        nc.sync.dma_start(out=wo_sb[:], in_=wo3[:, :, :])
        for g in (3, 4, 5, 6, 7, 0):
            qg_group(g)

        attn_stream(3, 0, split_at=32)
        attn_stream(3, 1)
        attn_stream(3, 2)
        attn_stream(3, 3)
        attn_stream(2, 0)
        attn_stream(2, 1)
        attn_stream(2, 2)
        pre = oproj_prefetch(3)
        attn_stream(2, 3)
        oproj_block(3, pre)
        attn_stream(1, 0)
        attn_stream(1, 1)
        attn_stream(1, 2)
        pre = oproj_prefetch(2)
        attn_stream(1, 3)
        oproj_block(2, pre)
        attn_stream(0, 0)
        attn_stream(0, 1)
        attn_stream(0, 2)
        pre = oproj_prefetch(1)
        attn_stream(0, 3)
        flush_finalize()  # fires AG(0,3) immediately; oproj(1) hides it
        oproj_block(1, pre)
        oproj_block0()

    nc.compile()
    return nc


def _tile_hid(a):
    """[HID, C] -> [128, NHD, C] with (p, k, c) = a[k*128+p, c]."""
    return np.ascontiguousarray(
        a.reshape(NHD, 128, a.shape[1]).transpose(1, 0, 2)).astype(bfloat16)


def _host_prep(context, query, w_qkv, w_o, q_norm_w, k_norm_w):
    context = np.asarray(context, dtype=np.float32)
    query = np.asarray(query, dtype=np.float32)
    w_qkv = np.asarray(w_qkv, dtype=np.float32)
    w_o = np.asarray(w_o, dtype=np.float32)
    q_norm_w = np.asarray(q_norm_w, dtype=np.float32)
    k_norm_w = np.asarray(k_norm_w, dtype=np.float32)

    ck = np.concatenate([context, query], axis=0)  # [KV, HID]
    ckT = np.ascontiguousarray(ck.T)  # [HID, KV] f32
    # [g][p][k][c] blocks of 256 kv columns (one projection group each)
    ckT4 = np.ascontiguousarray(
        ckT.reshape(NHD, 128, KV // 256, 256).transpose(2, 1, 0, 3)
    ).astype(bfloat16)

    wq = w_qkv[:, :H * D]
    wk = w_qkv[:, H * D:H * D + KVH * D]
    wv = w_qkv[:, H * D + KVH * D:]

    half = D // 2
    inv_freq = (1.0 / (THETA ** (np.arange(0, half, dtype=np.float32) / half))
                ).astype(np.float32)
    pos = np.arange(KV, dtype=np.float32)
    freqs = pos[:, None] * inv_freq[None, :]
    cosf, sinf = np.cos(freqs), np.sin(freqs)  # [KV, 64]

    def cs4(w):
        # [cos*w1 | sin*w2 | cos*w2 | sin*w1], w1 = w[:64], w2 = w[64:]
        return np.concatenate([cosf * w[None, :half], sinf * w[None, half:],
                               cosf * w[None, half:], sinf * w[None, :half]],
                              axis=1).astype(np.float32)  # [KV, 2D]

    cs4q = cs4(q_norm_w)[CTX:]  # [QL, 2D]
    cs4k = cs4(k_norm_w)  # [KV, 2D]
    csq_t = np.ascontiguousarray(
        cs4q.reshape(NQC, 128, 2 * D).transpose(1, 0, 2)).astype(bfloat16)
    csk_t = np.ascontiguousarray(
        cs4k.reshape(NKV, 128, 2 * D).transpose(1, 0, 2)).astype(bfloat16)

    p = np.arange(128)[:, None]
    t = np.arange(896)[None, :]
    msk = np.where(p <= t - 384, 0.0, MASKVAL).astype(bfloat16)  # [128, 896]

    in_maps = []
    for c in range(NCORES):
        in_maps.append({
            "ckT4": ckT4,
            "wq3": _tile_hid(wq[:, c * HPC * D:(c + 1) * HPC * D]),
            "wkv3": _tile_hid(np.concatenate(
                [wk[:, c * D:(c + 1) * D], wv[:, c * D:(c + 1) * D]], axis=1)),
            "wo3": _tile_hid(w_o[:, c * HPC * D:(c + 1) * HPC * D]),
            "csq": csq_t,
            "csk": csk_t,
            "msk": msk,
        })
    return in_maps


def kernel(context, query, w_qkv, w_o, q_norm_w, k_norm_w, **kw):
    if "nc" not in _STATE:
        _STATE["nc"] = _build()
    nc = _STATE["nc"]
    in_maps = _host_prep(context, query, w_qkv, w_o, q_norm_w, k_norm_w)
    res = run_bass_kernel_spmd(nc, in_maps, list(range(NCORES)), **kw)
    out = np.concatenate([np.asarray(res.results[c]["out"]) for c in range(NCORES)],
                         axis=1)
    if kw:
        return out.astype(np.float32), res
    return out.astype(np.float32)


# revision 32
# speedup vs baseline: 1.3731x; 1.0443x over previous
"""DFlash Qwen3 cross-attention on 8 TRN2 NeuronCores (v3).

Sharding: tensor-parallel over heads. Core c owns KV head c (KVH=8) and the
4 query heads 4c..4c+3 of its GQA group. Each core computes its heads'
QKV projections, per-head RMSNorm + RoPE, causal attention; per q-tile j the
normalized attention outputs (transposed, [4*D, 512]) are AllGathered so
every core holds attn^T for all 32 heads; each core computes a 512-column
slice of o_proj and the host concatenates the 8 slices.

v3 structure (vs v2, driven by the NTFF profile of v2):
- phase order: context proj (16 groups) -> query proj (groups 1..7 then 0)
  -> attention j=3,2,1,0 -> o_proj j' trailing one attention block. The last
  attention block is the cheapest (j=0), and o_proj(1) hides AG(0) latency.
- bubble-free attention quads: both score-pair matmuls emitted before the
  exps, AV matmuls after, so the PE never waits a full exp latency mid-quad.
- stream finalize (denominator matmuls, reciprocal, normalize, AllGather
  payload write) deferred into the NEXT stream after its first quad: kills
  the ~2.8us PE stall + HAM re-throttle at every stream boundary.
- per-j single AllGather with a [128, 4(qc), 4(h), 128] payload so o_proj
  reads back [128, 8c, 4h, 128] tiles as 1KB-contiguous segments, prefetched
  double-buffered on the idle sync DMA queue.
- RMSNorm rsqrt fully on DVE (quake bit-trick + 1 Newton step): the ACT
  engine runs exp only -- no ACT table swaps (v2 paid 27 x 1.3us mid-attention).
- wkv weight DMA split in 4 + non-critical loads (wq/wo/msk) deferred past
  the startup barrier window so the first matmul issues ~40us earlier.
"""

from contextlib import ExitStack

import numpy as np
from ml_dtypes import bfloat16

import concourse.bass as bass
import concourse.bass_isa as bass_isa
import concourse.mybir as mybir
import concourse.tile as tile
from concourse import bacc
from concourse.bass_utils import run_bass_kernel_spmd
from concourse.masks import make_identity

H = 32
KVH = 8
D = 128
HID = 4096
CTX = 4096
QL = 2048
KV = CTX + QL  # 6144
NCORES = 8
HPC = H // NCORES  # 4 q heads per core
THETA = 1000000.0
EPS = 1e-6
SCALE = float(D) ** -0.5

NHD = HID // 128  # 32 contraction chunks
NKV = KV // 128  # 48 kv chunks
NQC = QL // 128  # 16 q row chunks
NQJ = QL // 512  # 4 q column tiles for attention
NCTX = CTX // 128  # 32 context kv chunks
MASKVAL = -1e6
MAGIC = 0x5F3759DF  # fast inverse sqrt seed
SCHR_A = (128.0 / np.log(2.0)) * SCALE  # bf16 Schraudolph exp slope
SCHR_B = 127.0 * 128.0 - 7.5  # bf16 Schraudolph exp bias (tuned offline)

F32 = mybir.dt.float32
BF16 = mybir.dt.bfloat16
I32 = mybir.dt.int32
ALU = mybir.AluOpType

_STATE = {}


def _build():
    nc = bacc.Bacc()

    # ck^T in 256-kv-column blocks, [g][p][k][c] so one group load is an
    # 8KB-contiguous segment per partition (cheap descriptors, full HBM bw).
    ckT4 = nc.declare_dram_parameter("ckT4", [KV // 256, 128, NHD, 256], BF16,
                                     isOutput=False)
    wq3 = nc.declare_dram_parameter("wq3", [128, NHD, HPC * D], BF16, isOutput=False)
    wkv3 = nc.declare_dram_parameter("wkv3", [128, NHD, 2 * D], BF16, isOutput=False)
    wo3 = nc.declare_dram_parameter("wo3", [128, NHD, HPC * D], BF16, isOutput=False)
    csq = nc.declare_dram_parameter("csq", [128, NQC, 2 * D], BF16, isOutput=False)
    csk = nc.declare_dram_parameter("csk", [128, NKV, 2 * D], BF16, isOutput=False)
    mskd = nc.declare_dram_parameter("msk", [128, 896], BF16, isOutput=False)
    out_ext = nc.declare_dram_parameter("out", [QL, HPC * D], F32, isOutput=True)

    warm_in = nc.dram_tensor("warm_in", [128, 8], BF16)
    warm_out = nc.dram_tensor("warm_out", [NCORES * 128, 8], BF16, addr_space="Shared")
    # AllGather payload per j: [128 d-part, 4 qc, 4 h, 128 q] so the o_proj
    # readback for one qc is 1KB-contiguous per core block. j=0 (the LAST
    # attention block) gathers per head instead so only a 1MB collective and
    # the h=3 quarter of o_proj(0) remain on the critical tail.
    ag_ins = [nc.dram_tensor(f"ag_in{j}", [128, 4, HPC, 128], BF16)
              for j in range(1, NQJ)]
    ag_outs = [nc.dram_tensor(f"ag_out{j}", [NCORES * 128, 4, HPC, 128], BF16,
                              addr_space="Shared") for j in range(1, NQJ)]
    ag0_ins = [nc.dram_tensor(f"ag0_in{h}", [128, 4, 128], BF16)
               for h in range(HPC)]
    ag0_outs = [nc.dram_tensor(f"ag0_out{h}", [NCORES * 128, 4, 128], BF16,
                               addr_space="Shared") for h in range(HPC)]

    with tile.TileContext(nc) as tc, ExitStack() as ctx:
        singles = ctx.enter_context(tc.tile_pool(name="singles", bufs=1))
        ckv_pool = ctx.enter_context(tc.tile_pool(name="ckv", bufs=3))
        evac_pool = ctx.enter_context(tc.tile_pool(name="evac", bufs=3))
        work_pool = ctx.enter_context(tc.tile_pool(name="work", bufs=2))
        pt_pool = ctx.enter_context(tc.tile_pool(name="ptp", bufs=3))
        saccs_pool = ctx.enter_context(tc.tile_pool(name="sac", bufs=2))
        stg_pool = ctx.enter_context(tc.tile_pool(name="stg", bufs=2))
        at_pool = ctx.enter_context(tc.tile_pool(name="atp", bufs=2))
        # PSUM: 4 x 1-bank accumulators + 2 x 2-bank score tiles = 8 banks
        p1 = ctx.enter_context(tc.tile_pool(name="p1", bufs=4, space="PSUM"))
        p2 = ctx.enter_context(tc.tile_pool(name="p2", bufs=2, space="PSUM"))

        # ---- critical-path DMAs first: wkv (split x4) + first ck chunks ----
        wkv_sb = singles.tile([128, NHD, 2 * D], BF16)
        for q4 in range(4):
            nc.scalar.dma_start(out=wkv_sb[:, q4 * 8:(q4 + 1) * 8, :],
                                in_=wkv3[:, q4 * 8:(q4 + 1) * 8, :])

        def load_ckv(c0):
            """Stream ck^T block c0//256 in two k-halves (sync queue, so
            the warmup collective trigger on gpsimd can't block them)."""
            g2 = c0 // 256
            ca = ckv_pool.tile([128, NHD // 2, 256], BF16, tag="ckv")
            nc.sync.dma_start(out=ca, in_=ckT4[g2, :, 0:NHD // 2, :])
            cb = ckv_pool.tile([128, NHD // 2, 256], BF16, tag="ckv")
            nc.sync.dma_start(out=cb, in_=ckT4[g2, :, NHD // 2:NHD, :])

            def sl(k, c):
                t = ca if k < NHD // 2 else cb
                return t[:, k % (NHD // 2), c * 128:(c + 1) * 128]
            return sl

        # ---- warmup collective: pay the first-collective handshake now ----
        wtile = singles.tile([128, 8], BF16)
        nc.vector.memset(wtile, 0.0)
        nc.gpsimd.dma_start(out=warm_in[:, :], in_=wtile)
        nc.gpsimd.collective_compute(
            "AllGather", ALU.bypass, ins=[warm_in[:, :]], outs=[warm_out[:, :]],
            replica_groups=[list(range(NCORES))])

        # ---- constants ----
        identb = singles.tile([128, 128], BF16)
        make_identity(nc, identb)
        ones_bf = singles.tile([128, 128], BF16)
        nc.vector.memset(ones_bf, 1.0)
        zbias = singles.tile([128, 1], F32)
        nc.vector.memset(zbias, 0.0)
        magict = singles.tile([128, 4], I32)
        nc.vector.memset(magict, MAGIC)

        wq_sb = singles.tile([128, NHD, HPC * D], BF16)  # loaded mid-cg
        # compact causal mask: msk_sb[p, 384-128i + q] = 0 if 128i+p <= q
        # else MASKVAL, so chunk i uses the slice [384-128i : 896-128i].
        msk_sb = singles.tile([128, 896], BF16)  # loaded after cg0
        wo_sb = singles.tile([128, NHD, HPC * D], BF16)  # loaded mid-qg

        qT_sb = singles.tile([128, HPC, QL], BF16)  # Q^T per head: [d, h, q]
        kT_sb = singles.tile([128, KV], BF16)  # K^T: [d, kv]
        v_sb = singles.tile([128, NKV, D], BF16)  # V: [kv%128, r, d]

        def rsqrt_sum(rr, ssum, n, tg):
            """rr = 1/sqrt(ssum/D + eps), all on DVE (no ACT table swap)."""
            x = work_pool.tile([128, n], F32, tag=f"rx{tg}")
            nc.vector.tensor_scalar(out=x, in0=ssum, scalar1=1.0 / D,
                                    scalar2=EPS, op0=ALU.mult, op1=ALU.add)
            yi = work_pool.tile([128, n], I32, tag=f"ry{tg}")
            nc.vector.tensor_scalar(out=yi, in0=x[:, :].bitcast(I32), scalar1=1,
                                    scalar2=None, op0=ALU.logical_shift_right)
            nc.vector.tensor_tensor(out=yi, in0=magict[:, 0:n], in1=yi,
                                    op=ALU.subtract)
            y = yi[:, :].bitcast(F32)
            t1 = work_pool.tile([128, n], F32, tag=f"rt{tg}")
            nc.vector.tensor_tensor(out=t1, in0=y, in1=y, op=ALU.mult)
            nc.vector.tensor_tensor(out=t1, in0=t1, in1=x, op=ALU.mult)
            nc.vector.tensor_scalar(out=t1, in0=t1, scalar1=-0.5, scalar2=1.5,
                                    op0=ALU.mult, op1=ALU.add)
            nc.vector.tensor_tensor(out=rr, in0=t1, in1=y, op=ALU.mult)

        def rope(ro, xn, cs, t1):
            """ro = rotate-half(xn) with cs = [cw1|sw2|cw2|sw1] slices."""
            hf = D // 2
            nc.vector.tensor_mul(ro[:, :, 0:hf], xn[:, :, 0:hf], cs[0])
            nc.vector.tensor_mul(t1, xn[:, :, hf:D], cs[1])
            nc.vector.tensor_sub(ro[:, :, 0:hf], ro[:, :, 0:hf], t1)
            nc.vector.tensor_mul(ro[:, :, hf:D], xn[:, :, hf:D], cs[2])
            nc.vector.tensor_mul(t1, xn[:, :, 0:hf], cs[3])
            nc.vector.tensor_add(ro[:, :, hf:D], ro[:, :, hf:D], t1)

        def qnorm(qe, qc, csqp, ci):
            """qe: [128, 4, 128] bf16 SBUF evac of the q projection.
            norm+rope -> 4 transposes -> qT_sb."""
            sq = work_pool.tile([128, HPC, D], BF16, tag="qsq")
            nc.vector.tensor_mul(sq, qe, qe)
            ssum = work_pool.tile([128, HPC], F32, tag="qssum")
            nc.vector.tensor_reduce(ssum, sq, axis=mybir.AxisListType.X, op=ALU.add)
            rr = work_pool.tile([128, HPC], F32, tag="qrr")
            rsqrt_sum(rr, ssum, HPC, "q")
            qn = work_pool.tile([128, HPC, D], BF16, tag="qn")
            for h in range(HPC):
                nc.vector.tensor_scalar_mul(out=qn[:, h, :], in0=qe[:, h, :],
                                            scalar1=rr[:, h:h + 1])
            hf = D // 2
            ro = work_pool.tile([128, HPC, D], BF16, tag="qro")
            t1 = work_pool.tile([128, HPC, hf], BF16, tag="qt1")
            for h in range(HPC):
                cs = [csqp[:, ci:ci + 1, s * hf:(s + 1) * hf] for s in range(4)]
                rope(ro[:, h:h + 1, :], qn[:, h:h + 1, :], cs, t1[:, 0:1, :])
            for h in range(HPC):
                tp = p1.tile([128, 128], BF16, tag="acc", name=f"tpq{qc}_{h}")
                nc.tensor.transpose(tp, ro[:, h, :], identb)
                nc.scalar.copy(out=qT_sb[:, h, qc * 128:(qc + 1) * 128], in_=tp)

        def knorm_pair(ke, r0, cskp):
            """ke: [128, 2, 2D] bf16 SBUF evac of the K|V projection pair."""
            sq = work_pool.tile([128, 2, D], BF16, tag="ksq")
            nc.vector.tensor_mul(sq, ke[:, :, 0:D], ke[:, :, 0:D])
            ssum = work_pool.tile([128, 2], F32, tag="kssum")
            nc.vector.tensor_reduce(ssum, sq, axis=mybir.AxisListType.X, op=ALU.add)
            rr = work_pool.tile([128, 2], F32, tag="krr")
            rsqrt_sum(rr, ssum, 2, "k")
            kn = work_pool.tile([128, 2, D], BF16, tag="kn")
            for c in range(2):
                nc.vector.tensor_scalar_mul(out=kn[:, c, :], in0=ke[:, c, 0:D],
                                            scalar1=rr[:, c:c + 1])
            hf = D // 2
            cs = [cskp[:, :, s * hf:(s + 1) * hf] for s in range(4)]
            ro = work_pool.tile([128, 2, D], BF16, tag="kro")
            t1 = work_pool.tile([128, 2, hf], BF16, tag="kt1")
            rope(ro, kn, cs, t1)
            for c in range(2):
                tp = p1.tile([128, 128], BF16, tag="acc", name=f"tpk{r0}_{c}")
                nc.tensor.transpose(tp, ro[:, c, :], identb)
                nc.scalar.copy(out=kT_sb[:, (r0 + c) * 128:(r0 + c + 1) * 128], in_=tp)
            nc.vector.tensor_copy(out=v_sb[:, r0:r0 + 2, :], in_=ke[:, :, D:2 * D])

        # Deferred-norm pipeline: each projection group's norm+rope+PE
        # transposes are emitted AFTER the NEXT group's matmuls so the DVE
        # chain hides behind tensor work.
        pending = []

        def flush_pending():
            while pending:
                pending.pop(0)()

        def cg_group(g):
            """Context projection: kv chunks 2g, 2g+1."""
            sl = load_ckv(g * 256)
            cskp = evac_pool.tile([128, 2, 2 * D], BF16, tag="cskp")
            nc.gpsimd.dma_start(out=cskp, in_=csk[:, 2 * g:2 * g + 2, :])
            pk = p1.tile([128, 2, 2 * D], F32, tag="acc", name=f"cgk{g}")
            for k in range(NHD):
                for c in range(2):
                    nc.tensor.matmul(pk[:, c, :], lhsT=sl(k, c),
                                     rhs=wkv_sb[:, k, :],
                                     start=(k == 0 and c == 0),
                                     stop=(k == NHD - 1))
            ke = evac_pool.tile([128, 2, 2 * D], BF16, tag="ke")
            nc.scalar.copy(out=ke[:], in_=pk)
            flush_pending()
            pending.append(lambda ke=ke, r0=2 * g, cskp=cskp: knorm_pair(ke, r0, cskp))

        def qg_group(g):
            """Query-row projection: kv chunks 32+2g, 33+2g (q chunks 2g,
            2g+1): shared stationary feeds both wkv and wq."""
            sl = load_ckv(CTX + g * 256)
            cskp = evac_pool.tile([128, 2, 2 * D], BF16, tag="cskp")
            nc.gpsimd.dma_start(out=cskp, in_=csk[:, NCTX + 2 * g:NCTX + 2 * g + 2, :])
            csqp = evac_pool.tile([128, 2, 2 * D], BF16, tag="csqp")
            nc.gpsimd.dma_start(out=csqp, in_=csq[:, 2 * g:2 * g + 2, :])
            pk = p1.tile([128, 2, 2 * D], F32, tag="acc", name=f"qgk{g}")
            pq = [p1.tile([128, HPC * D], F32, tag="acc", name=f"qgq{g}_{c}")
                  for c in range(2)]
            for k in range(NHD):
                for c in range(2):
                    st = sl(k, c)
                    nc.tensor.matmul(pk[:, c, :], lhsT=st, rhs=wkv_sb[:, k, :],
                                     start=(k == 0 and c == 0),
                                     stop=(k == NHD - 1))
                    nc.tensor.matmul(pq[c], lhsT=st, rhs=wq_sb[:, k, :],
                                     start=(k == 0), stop=(k == NHD - 1))
            ke = evac_pool.tile([128, 2, 2 * D], BF16, tag="ke")
            nc.scalar.copy(out=ke[:], in_=pk)
            qes = []
            for c in range(2):
                qe = evac_pool.tile([128, HPC, D], BF16, tag="qe")
                nc.scalar.copy(out=qe[:], in_=pq[c])
                qes.append(qe)
            flush_pending()

            def norm(ke=ke, qes=qes, csqp=csqp, cskp=cskp, g=g):
                knorm_pair(ke, NCTX + 2 * g, cskp)
                for c in range(2):
                    qnorm(qes[c], 2 * g + c, csqp, c)
            pending.append(norm)

        # ---- attention ----
        # finalize (den matmuls, reciprocal, normalize, AG payload write) of
        # the previous stream is deferred and emitted after the next block of
        # dense PE work has started.
        finalize_pending = []

        def flush_finalize():
            while finalize_pending:
                finalize_pending.pop(0)()

        def make_finalize(j, h, o_acc, saccs):
            def fin():
                den = p1.tile([128, 512], F32, tag="acc", name=f"den{j}_{h}")
                for s in range(4):
                    nc.tensor.matmul(den, lhsT=ones_bf,
                                     rhs=saccs[:, s * 512:(s + 1) * 512],
                                     start=(s == 0), stop=(s == 3))
                pr = work_pool.tile([128, 512], F32, tag="dps")
                nc.vector.reciprocal_approx_fast(out=pr, in_=den)
                s3 = stg_pool.tile([128, 512], BF16, tag="stg")
                nc.vector.tensor_mul(s3, o_acc, pr)
                s3v = s3[:, :].rearrange("p (c q) -> p c q", c=4)
                if j == 0:
                    nc.gpsimd.dma_start(out=ag0_ins[h][:, :, :], in_=s3v)
                    nc.gpsimd.collective_compute(
                        "AllGather", ALU.bypass, ins=[ag0_ins[h][:, :, :]],
                        outs=[ag0_outs[h][:, :, :]],
                        replica_groups=[list(range(NCORES))])
                else:
                    nc.gpsimd.dma_start(out=ag_ins[j - 1][:, :, h, :], in_=s3v)
                    if h == HPC - 1:
                        nc.gpsimd.collective_compute(
                            "AllGather", ALU.bypass,
                            ins=[ag_ins[j - 1][:, :, :, :]],
                            outs=[ag_outs[j - 1][:, :, :, :]],
                            replica_groups=[list(range(NCORES))])
            return fin

        def attn_stream(j, h, split_at=None):
            """One (j,h) attention stream, software-pipelined one quad deep:
            the AV matmuls of quad q are emitted after the score matmuls of
            quad q+1 so the exp latency hides under PE work.
            split_at: optional chunk index at which to flush the projection
            norm pipeline (used once, to hide the last qg group's norm)."""
            r_list = list(range(NCTX + 4 + 4 * j))
            nq = len(r_list) // 4
            qTj = qT_sb[:, h, j * 512:(j + 1) * 512]
            o_acc = p1.tile([128, 512], F32, tag="acc", name=f"o{j}_{h}")
            saccs = saccs_pool.tile([128, 4 * 512], BF16, tag="sacc")

            def front(qd):
                """Score matmuls (+ causal-mask PSUM pre-bias via identity
                matmul) and exp for quad qd. Returns the exp'd ptw tile."""
                ptw = pt_pool.tile([128, 4 * 512], BF16, tag="ptw")
                sts = []
                for pi in range(2):
                    rA = r_list[qd * 4 + 2 * pi]
                    rB = r_list[qd * 4 + 2 * pi + 1]
                    st = p2.tile([128, 1024], F32, tag="st")
                    for idx, r in ((0, rA), (1, rB)):
                        i = r - NCTX - 4 * j
                        diag = 0 <= i <= 3
                        if diag:
                            nc.tensor.matmul(
                                st[:, idx * 512:(idx + 1) * 512], lhsT=identb,
                                rhs=msk_sb[:, 384 - 128 * i:896 - 128 * i],
                                start=True, stop=False)
                        nc.tensor.matmul(st[:, idx * 512:(idx + 1) * 512],
                                         lhsT=kT_sb[:, r * 128:(r + 1) * 128],
                                         rhs=qTj, start=not diag, stop=True)
                    sts.append(st)
                for pi in range(2):
                    # Every 4th score pair (never a masked/diagonal one) goes
                    # through a bf16-Schraudolph exp on DVE instead of ACT:
                    # bf16_bits(exp(x*SCALE)) ~= int16(128/ln2*SCALE*x + B).
                    # The constant bias cancels in the softmax normalization;
                    # the ~1.8% rms sawtooth on 1/4 of the weights is well
                    # inside the error budget, and the ACT engine stops
                    # pacing the attention phase.
                    rB = r_list[qd * 4 + 2 * pi + 1]
                    if pi == 0 and qd % 2 == 0 and rB < NCTX:
                        nc.vector.tensor_scalar(
                            out=ptw[:, 0:1024].bitcast(mybir.dt.int16),
                            in0=sts[pi][:], scalar1=SCHR_A, scalar2=SCHR_B,
                            op0=ALU.mult, op1=ALU.add)
                    else:
                        nc.scalar.activation(
                            out=ptw[:, pi * 1024:(pi + 1) * 1024],
                            in_=sts[pi][:],
                            func=mybir.ActivationFunctionType.Exp,
                            bias=zbias, scale=SCALE)
                return ptw

            def back(qd, ptw):
                for slot in range(4):
                    r = r_list[qd * 4 + slot]
                    nc.tensor.matmul(
                        o_acc, lhsT=v_sb[:, r, :],
                        rhs=ptw[:, slot * 512:(slot + 1) * 512],
                        start=(qd == 0 and slot == 0),
                        stop=(qd == nq - 1 and slot == 3))
                if qd == 0:
                    nc.vector.tensor_copy(out=saccs, in_=ptw)
                else:
                    nc.vector.tensor_add(saccs, saccs, ptw)

            prev = None
            for qd in range(nq):
                if split_at is not None and qd * 4 == split_at:
                    flush_pending()
                ptw = front(qd)
                if qd == 1:
                    flush_finalize()
                if prev is not None:
                    back(qd - 1, prev)
                prev = ptw
            back(nq - 1, prev)
            finalize_pending.append(make_finalize(j, h, o_acc, saccs))

        def at_load(jq, ats, hv):
            qc2, lo = hv // 2, (hv % 2) * 4
            at = at_pool.tile([128, 4, HPC, 128], BF16, tag="at")
            nc.sync.dma_start(
                out=at,
                in_=ag_outs[jq - 1][lo * 128:(lo + 4) * 128, qc2, :, :]
                .rearrange("(c p) h q -> p c h q", p=128))
            ats[hv] = at

        def oproj_prefetch(jq):
            """Issue the first two attn^T tile loads for oproj_block(jq)
            early, so its first matmuls don't expose the DMA latency."""
            ats = {}
            at_load(jq, ats, 0)
            at_load(jq, ats, 1)
            return ats

        def oproj_block(jq, ats):
            """o_proj for q tile jq (jq >= 1) from the AllGathered attn^T.
            Tiles load per (qc, 4-core half) as 1KB-contiguous segments."""
            for qc2 in range(4):
                qc = 4 * jq + qc2
                po = p1.tile([128, HPC * D], F32, tag="acc", name=f"po{qc}")
                for half in range(2):
                    at = ats.pop(qc2 * 2 + half)
                    for c4 in range(4):
                        for h2 in range(HPC):
                            gg = (half * 4 + c4) * HPC + h2
                            nc.tensor.matmul(po, lhsT=at[:, c4, h2, :],
                                             rhs=wo_sb[:, gg, :],
                                             start=(gg == 0), stop=(gg == H - 1))
                    if qc2 == 0 and half == 0:
                        flush_finalize()
                    if qc2 * 2 + half + 2 < 8:
                        at_load(jq, ats, qc2 * 2 + half + 2)
                ot = stg_pool.tile([128, HPC * D], F32, tag="ot")
                nc.scalar.copy(out=ot, in_=po)
                nc.sync.dma_start(out=out_ext[qc * 128:(qc + 1) * 128, :], in_=ot)

        def oproj_block0():
            """o_proj for q tile 0, head-major: the h2<3 contraction chunks
            run while AG(0, h=3) is still in flight, so only the last-quarter
            matmuls trail the final collective."""
            pos = [p1.tile([128, HPC * D], F32, tag="acc", name=f"po0_{qc2}")
                   for qc2 in range(4)]
            ats = {}

            def at0_load(h2, qc2):
                at = at_pool.tile([128, NCORES, 128], BF16, tag="at")
                nc.sync.dma_start(
                    out=at,
                    in_=ag0_outs[h2][:, qc2, :].rearrange("(c p) q -> p c q",
                                                          p=128))
                ats[(h2, qc2)] = at

            at0_load(0, 0)
            at0_load(0, 1)
            order = [(h2, qc2) for h2 in range(HPC) for qc2 in range(4)]
            for oi, (h2, qc2) in enumerate(order):
                at = ats.pop((h2, qc2))
                po = pos[qc2]
                for cc in range(NCORES):
                    nc.tensor.matmul(po, lhsT=at[:, cc, :],
                                     rhs=wo_sb[:, cc * HPC + h2, :],
                                     start=(h2 == 0 and cc == 0),
                                     stop=(h2 == HPC - 1 and cc == NCORES - 1))
                if oi + 2 < len(order):
                    at0_load(*order[oi + 2])
            for qc2 in range(4):
                ot = stg_pool.tile([128, HPC * D], F32, tag="ot")
                nc.scalar.copy(out=ot, in_=pos[qc2])
                nc.sync.dma_start(out=out_ext[qc2 * 128:(qc2 + 1) * 128, :],
                                  in_=ot)

        # ---- main sequence ----
        cg_group(0)
        nc.sync.dma_start(out=msk_sb[:], in_=mskd[:, :])
        for g in range(1, 10):
            cg_group(g)
        nc.scalar.dma_start(out=wq_sb[:], in_=wq3[:, :, :])
        for g in range(10, 16):
            cg_group(g)
        # qg group 0 (q chunks 0,1 / kv 32,33) runs LAST so its deferred norm
        # can flush inside the first attention stream (which needs q chunks
        # 12-15 and all kv, but touches kv>=32 only from its 9th quad on).
        qg_group(1)
        qg_group(2)
        nc.sync.dma_start(out=wo_sb[:], in_=wo3[:, :, :])
        for g in (3, 4, 5, 6, 7, 0):
            qg_group(g)

        attn_stream(3, 0, split_at=32)
        attn_stream(3, 1)
        attn_stream(3, 2)
        attn_stream(3, 3)
        attn_stream(2, 0)
        attn_stream(2, 1)
        attn_stream(2, 2)
        pre = oproj_prefetch(3)
        attn_stream(2, 3)
        oproj_block(3, pre)
        attn_stream(1, 0)
        attn_stream(1, 1)
        attn_stream(1, 2)
        pre = oproj_prefetch(2)
        attn_stream(1, 3)
        oproj_block(2, pre)
        attn_stream(0, 0)
        attn_stream(0, 1)
        attn_stream(0, 2)
        pre = oproj_prefetch(1)
        attn_stream(0, 3)
        flush_finalize()  # fires AG(0,3) immediately; oproj(1) hides it
        oproj_block(1, pre)
        oproj_block0()

    nc.compile()
    return nc


def _tile_hid(a):
    """[HID, C] -> [128, NHD, C] with (p, k, c) = a[k*128+p, c]."""
    return np.ascontiguousarray(
        a.reshape(NHD, 128, a.shape[1]).transpose(1, 0, 2)).astype(bfloat16)


def _host_prep(context, query, w_qkv, w_o, q_norm_w, k_norm_w):
    context = np.asarray(context, dtype=np.float32)
    query = np.asarray(query, dtype=np.float32)
    w_qkv = np.asarray(w_qkv, dtype=np.float32)
    w_o = np.asarray(w_o, dtype=np.float32)
    q_norm_w = np.asarray(q_norm_w, dtype=np.float32)
    k_norm_w = np.asarray(k_norm_w, dtype=np.float32)

    ck = np.concatenate([context, query], axis=0)  # [KV, HID]
    ckT = np.ascontiguousarray(ck.T)  # [HID, KV] f32
    # [g][p][k][c] blocks of 256 kv columns (one projection group each)
    ckT4 = np.ascontiguousarray(
        ckT.reshape(NHD, 128, KV // 256, 256).transpose(2, 1, 0, 3)
    ).astype(bfloat16)

    wq = w_qkv[:, :H * D]
    wk = w_qkv[:, H * D:H * D + KVH * D]
    wv = w_qkv[:, H * D + KVH * D:]

    half = D // 2
    inv_freq = (1.0 / (THETA ** (np.arange(0, half, dtype=np.float32) / half))
                ).astype(np.float32)
    pos = np.arange(KV, dtype=np.float32)
    freqs = pos[:, None] * inv_freq[None, :]
    cosf, sinf = np.cos(freqs), np.sin(freqs)  # [KV, 64]

    def cs4(w):
        # [cos*w1 | sin*w2 | cos*w2 | sin*w1], w1 = w[:64], w2 = w[64:]
        return np.concatenate([cosf * w[None, :half], sinf * w[None, half:],
                               cosf * w[None, half:], sinf * w[None, :half]],
                              axis=1).astype(np.float32)  # [KV, 2D]

    cs4q = cs4(q_norm_w)[CTX:]  # [QL, 2D]
    cs4k = cs4(k_norm_w)  # [KV, 2D]
    csq_t = np.ascontiguousarray(
        cs4q.reshape(NQC, 128, 2 * D).transpose(1, 0, 2)).astype(bfloat16)
    csk_t = np.ascontiguousarray(
        cs4k.reshape(NKV, 128, 2 * D).transpose(1, 0, 2)).astype(bfloat16)

    p = np.arange(128)[:, None]
    t = np.arange(896)[None, :]
    msk = np.where(p <= t - 384, 0.0, MASKVAL).astype(bfloat16)  # [128, 896]

    in_maps = []
    for c in range(NCORES):
        in_maps.append({
            "ckT4": ckT4,
            "wq3": _tile_hid(wq[:, c * HPC * D:(c + 1) * HPC * D]),
            "wkv3": _tile_hid(np.concatenate(
                [wk[:, c * D:(c + 1) * D], wv[:, c * D:(c + 1) * D]], axis=1)),
            "wo3": _tile_hid(w_o[:, c * HPC * D:(c + 1) * HPC * D]),
            "csq": csq_t,
            "csk": csk_t,
            "msk": msk,
        })
    return in_maps


def kernel(context, query, w_qkv, w_o, q_norm_w, k_norm_w, **kw):
    if "nc" not in _STATE:
        _STATE["nc"] = _build()
    nc = _STATE["nc"]
    in_maps = _host_prep(context, query, w_qkv, w_o, q_norm_w, k_norm_w)
    res = run_bass_kernel_spmd(nc, in_maps, list(range(NCORES)), **kw)
    out = np.concatenate([np.asarray(res.results[c]["out"]) for c in range(NCORES)],
                         axis=1)
    if kw:
        return out.astype(np.float32), res
    return out.astype(np.float32)


# revision 35
# speedup vs baseline: 1.3993x; 1.0191x over previous
"""DFlash Qwen3 cross-attention on 8 TRN2 NeuronCores (v3).

Sharding: tensor-parallel over heads. Core c owns KV head c (KVH=8) and the
4 query heads 4c..4c+3 of its GQA group. Each core computes its heads'
QKV projections, per-head RMSNorm + RoPE, causal attention; per q-tile j the
normalized attention outputs (transposed, [4*D, 512]) are AllGathered so
every core holds attn^T for all 32 heads; each core computes a 512-column
slice of o_proj and the host concatenates the 8 slices.

v3 structure (vs v2, driven by the NTFF profile of v2):
- phase order: context proj (16 groups) -> query proj (groups 1..7 then 0)
  -> attention j=3,2,1,0 -> o_proj j' trailing one attention block. The last
  attention block is the cheapest (j=0), and o_proj(1) hides AG(0) latency.
- bubble-free attention quads: both score-pair matmuls emitted before the
  exps, AV matmuls after, so the PE never waits a full exp latency mid-quad.
- stream finalize (denominator matmuls, reciprocal, normalize, AllGather
  payload write) deferred into the NEXT stream after its first quad: kills
  the ~2.8us PE stall + HAM re-throttle at every stream boundary.
- per-j single AllGather with a [128, 4(qc), 4(h), 128] payload so o_proj
  reads back [128, 8c, 4h, 128] tiles as 1KB-contiguous segments, prefetched
  double-buffered on the idle sync DMA queue.
- RMSNorm rsqrt fully on DVE (quake bit-trick + 1 Newton step): the ACT
  engine runs exp only -- no ACT table swaps (v2 paid 27 x 1.3us mid-attention).
- wkv weight DMA split in 4 + non-critical loads (wq/wo/msk) deferred past
  the startup barrier window so the first matmul issues ~40us earlier.
"""

from contextlib import ExitStack

import numpy as np
from ml_dtypes import bfloat16

import concourse.bass as bass
import concourse.bass_isa as bass_isa
import concourse.mybir as mybir
import concourse.tile as tile
from concourse import bacc
from concourse.bass_utils import run_bass_kernel_spmd
from concourse.masks import make_identity

H = 32
KVH = 8
D = 128
HID = 4096
CTX = 4096
QL = 2048
KV = CTX + QL  # 6144
NCORES = 8
HPC = H // NCORES  # 4 q heads per core
THETA = 1000000.0
EPS = 1e-6
SCALE = float(D) ** -0.5

NHD = HID // 128  # 32 contraction chunks
NKV = KV // 128  # 48 kv chunks
NQC = QL // 128  # 16 q row chunks
NQJ = QL // 512  # 4 q column tiles for attention
NCTX = CTX // 128  # 32 context kv chunks
MASKVAL = -1e6
MAGIC = 0x5F3759DF  # fast inverse sqrt seed
SCHR_A = (128.0 / np.log(2.0)) * SCALE  # bf16 Schraudolph exp slope
SCHR_B = 127.0 * 128.0 - 7.5  # bf16 Schraudolph exp bias (tuned offline)

F32 = mybir.dt.float32
BF16 = mybir.dt.bfloat16
I32 = mybir.dt.int32
ALU = mybir.AluOpType

_STATE = {}


def _build():
    nc = bacc.Bacc()

    # ck^T in 256-kv-column blocks, [g][p][k][c] so one group load is an
    # 8KB-contiguous segment per partition (cheap descriptors, full HBM bw).
    ckT4 = nc.declare_dram_parameter("ckT4", [KV // 256, 128, NHD, 256], BF16,
                                     isOutput=False)
    wq3 = nc.declare_dram_parameter("wq3", [128, NHD, HPC * D], BF16, isOutput=False)
    wkv3 = nc.declare_dram_parameter("wkv3", [128, NHD, 2 * D], BF16, isOutput=False)
    wo3 = nc.declare_dram_parameter("wo3", [128, NHD, HPC * D], BF16, isOutput=False)
    csq = nc.declare_dram_parameter("csq", [128, NQC, 2 * D], BF16, isOutput=False)
    csk = nc.declare_dram_parameter("csk", [128, NKV, 2 * D], BF16, isOutput=False)
    mskd = nc.declare_dram_parameter("msk", [128, 896], BF16, isOutput=False)
    out_ext = nc.declare_dram_parameter("out", [QL, HPC * D], F32, isOutput=True)

    warm_in = nc.dram_tensor("warm_in", [128, 8], BF16)
    warm_out = nc.dram_tensor("warm_out", [NCORES * 128, 8], BF16, addr_space="Shared")
    # AllGather payload per j: [128 d-part, 4 qc, 4 h, 128 q] so the o_proj
    # readback for one qc is 1KB-contiguous per core block. j=0 (the LAST
    # attention block) gathers per head instead so only a 1MB collective and
    # the h=3 quarter of o_proj(0) remain on the critical tail.
    ag_ins = [nc.dram_tensor(f"ag_in{j}", [128, 4, HPC, 128], BF16)
              for j in range(1, NQJ)]
    ag_outs = [nc.dram_tensor(f"ag_out{j}", [NCORES * 128, 4, HPC, 128], BF16,
                              addr_space="Shared") for j in range(1, NQJ)]
    ag0_ins = [nc.dram_tensor(f"ag0_in{h}", [128, 4, 128], BF16)
               for h in range(HPC)]
    ag0_outs = [nc.dram_tensor(f"ag0_out{h}", [NCORES * 128, 4, 128], BF16,
                               addr_space="Shared") for h in range(HPC)]

    with tile.TileContext(nc) as tc, ExitStack() as ctx:
        singles = ctx.enter_context(tc.tile_pool(name="singles", bufs=1))
        ckv_pool = ctx.enter_context(tc.tile_pool(name="ckv", bufs=3))
        evac_pool = ctx.enter_context(tc.tile_pool(name="evac", bufs=3))
        work_pool = ctx.enter_context(tc.tile_pool(name="work", bufs=2))
        pt_pool = ctx.enter_context(tc.tile_pool(name="ptp", bufs=3))
        saccs_pool = ctx.enter_context(tc.tile_pool(name="sac", bufs=2))
        stg_pool = ctx.enter_context(tc.tile_pool(name="stg", bufs=2))
        at_pool = ctx.enter_context(tc.tile_pool(name="atp", bufs=2))
        # PSUM: 4 x 1-bank accumulators + 2 x 2-bank score tiles = 8 banks
        p1 = ctx.enter_context(tc.tile_pool(name="p1", bufs=4, space="PSUM"))
        p2 = ctx.enter_context(tc.tile_pool(name="p2", bufs=2, space="PSUM"))

        # ---- critical-path DMAs first: wkv (split x4) + first ck chunks ----
        wkv_sb = singles.tile([128, NHD, 2 * D], BF16)
        for q4 in range(4):
            nc.scalar.dma_start(out=wkv_sb[:, q4 * 8:(q4 + 1) * 8, :],
                                in_=wkv3[:, q4 * 8:(q4 + 1) * 8, :])

        def load_ckv(c0):
            """Stream ck^T block c0//256 in two k-halves (sync queue, so
            the warmup collective trigger on gpsimd can't block them)."""
            g2 = c0 // 256
            ca = ckv_pool.tile([128, NHD // 2, 256], BF16, tag="ckv")
            nc.sync.dma_start(out=ca, in_=ckT4[g2, :, 0:NHD // 2, :])
            cb = ckv_pool.tile([128, NHD // 2, 256], BF16, tag="ckv")
            nc.sync.dma_start(out=cb, in_=ckT4[g2, :, NHD // 2:NHD, :])

            def sl(k, c):
                t = ca if k < NHD // 2 else cb
                return t[:, k % (NHD // 2), c * 128:(c + 1) * 128]
            return sl

        # ---- warmup collective: pay the first-collective handshake now ----
        wtile = singles.tile([128, 8], BF16)
        nc.vector.memset(wtile, 0.0)
        nc.gpsimd.dma_start(out=warm_in[:, :], in_=wtile)
        nc.gpsimd.collective_compute(
            "AllGather", ALU.bypass, ins=[warm_in[:, :]], outs=[warm_out[:, :]],
            replica_groups=[list(range(NCORES))])

        # ---- constants ----
        identb = singles.tile([128, 128], BF16)
        make_identity(nc, identb)
        ones_bf = singles.tile([128, 128], BF16)
        nc.vector.memset(ones_bf, 1.0)
        zbias = singles.tile([128, 1], F32)
        nc.vector.memset(zbias, 0.0)
        magict = singles.tile([128, 4], I32)
        nc.vector.memset(magict, MAGIC)

        wq_sb = singles.tile([128, NHD, HPC * D], BF16)  # loaded mid-cg
        # compact causal mask: msk_sb[p, 384-128i + q] = 0 if 128i+p <= q
        # else MASKVAL, so chunk i uses the slice [384-128i : 896-128i].
        msk_sb = singles.tile([128, 896], BF16)  # loaded after cg0
        wo_sb = singles.tile([128, NHD, HPC * D], BF16)  # loaded mid-qg

        qT_sb = singles.tile([128, HPC, QL], BF16)  # Q^T per head: [d, h, q]
        kT_sb = singles.tile([128, KV], BF16)  # K^T: [d, kv]
        v_sb = singles.tile([128, NKV, D], BF16)  # V: [kv%128, r, d]

        def rsqrt_sum(rr, ssum, n, tg):
            """rr = 1/sqrt(ssum/D + eps), all on DVE (no ACT table swap)."""
            x = work_pool.tile([128, n], F32, tag=f"rx{tg}")
            nc.vector.tensor_scalar(out=x, in0=ssum, scalar1=1.0 / D,
                                    scalar2=EPS, op0=ALU.mult, op1=ALU.add)
            yi = work_pool.tile([128, n], I32, tag=f"ry{tg}")
            nc.vector.tensor_scalar(out=yi, in0=x[:, :].bitcast(I32), scalar1=1,
                                    scalar2=None, op0=ALU.logical_shift_right)
            nc.vector.tensor_tensor(out=yi, in0=magict[:, 0:n], in1=yi,
                                    op=ALU.subtract)
            y = yi[:, :].bitcast(F32)
            t1 = work_pool.tile([128, n], F32, tag=f"rt{tg}")
            nc.vector.tensor_tensor(out=t1, in0=y, in1=y, op=ALU.mult)
            nc.vector.tensor_tensor(out=t1, in0=t1, in1=x, op=ALU.mult)
            nc.vector.tensor_scalar(out=t1, in0=t1, scalar1=-0.5, scalar2=1.5,
                                    op0=ALU.mult, op1=ALU.add)
            nc.vector.tensor_tensor(out=rr, in0=t1, in1=y, op=ALU.mult)

        def rope(ro, xn, cs, t1):
            """ro = rotate-half(xn) with cs = [cw1|sw2|cw2|sw1] slices."""
            hf = D // 2
            nc.vector.tensor_mul(ro[:, :, 0:hf], xn[:, :, 0:hf], cs[0])
            nc.vector.tensor_mul(t1, xn[:, :, hf:D], cs[1])
            nc.vector.tensor_sub(ro[:, :, 0:hf], ro[:, :, 0:hf], t1)
            nc.vector.tensor_mul(ro[:, :, hf:D], xn[:, :, hf:D], cs[2])
            nc.vector.tensor_mul(t1, xn[:, :, 0:hf], cs[3])
            nc.vector.tensor_add(ro[:, :, hf:D], ro[:, :, hf:D], t1)

        def qnorm(qe, qc, csqp, ci):
            """qe: [128, 4, 128] bf16 SBUF evac of the q projection.
            norm+rope -> 4 transposes -> qT_sb."""
            sq = work_pool.tile([128, HPC, D], BF16, tag="qsq")
            nc.vector.tensor_mul(sq, qe, qe)
            ssum = work_pool.tile([128, HPC], F32, tag="qssum")
            nc.vector.tensor_reduce(ssum, sq, axis=mybir.AxisListType.X, op=ALU.add)
            rr = work_pool.tile([128, HPC], F32, tag="qrr")
            rsqrt_sum(rr, ssum, HPC, "q")
            qn = work_pool.tile([128, HPC, D], BF16, tag="qn")
            for h in range(HPC):
                nc.vector.tensor_scalar_mul(out=qn[:, h, :], in0=qe[:, h, :],
                                            scalar1=rr[:, h:h + 1])
            hf = D // 2
            ro = work_pool.tile([128, HPC, D], BF16, tag="qro")
            t1 = work_pool.tile([128, HPC, hf], BF16, tag="qt1")
            for h in range(HPC):
                cs = [csqp[:, ci:ci + 1, s * hf:(s + 1) * hf] for s in range(4)]
                rope(ro[:, h:h + 1, :], qn[:, h:h + 1, :], cs, t1[:, 0:1, :])
            for h in range(HPC):
                tp = p1.tile([128, 128], BF16, tag="acc", name=f"tpq{qc}_{h}")
                nc.tensor.transpose(tp, ro[:, h, :], identb)
                nc.scalar.copy(out=qT_sb[:, h, qc * 128:(qc + 1) * 128], in_=tp)

        def knorm_pair(ke, r0, cskp):
            """ke: [128, 2, 2D] bf16 SBUF evac of the K|V projection pair."""
            sq = work_pool.tile([128, 2, D], BF16, tag="ksq")
            nc.vector.tensor_mul(sq, ke[:, :, 0:D], ke[:, :, 0:D])
            ssum = work_pool.tile([128, 2], F32, tag="kssum")
            nc.vector.tensor_reduce(ssum, sq, axis=mybir.AxisListType.X, op=ALU.add)
            rr = work_pool.tile([128, 2], F32, tag="krr")
            rsqrt_sum(rr, ssum, 2, "k")
            kn = work_pool.tile([128, 2, D], BF16, tag="kn")
            for c in range(2):
                nc.vector.tensor_scalar_mul(out=kn[:, c, :], in0=ke[:, c, 0:D],
                                            scalar1=rr[:, c:c + 1])
            hf = D // 2
            cs = [cskp[:, :, s * hf:(s + 1) * hf] for s in range(4)]
            ro = work_pool.tile([128, 2, D], BF16, tag="kro")
            t1 = work_pool.tile([128, 2, hf], BF16, tag="kt1")
            rope(ro, kn, cs, t1)
            for c in range(2):
                tp = p1.tile([128, 128], BF16, tag="acc", name=f"tpk{r0}_{c}")
                nc.tensor.transpose(tp, ro[:, c, :], identb)
                nc.scalar.copy(out=kT_sb[:, (r0 + c) * 128:(r0 + c + 1) * 128], in_=tp)
            nc.vector.tensor_copy(out=v_sb[:, r0:r0 + 2, :], in_=ke[:, :, D:2 * D])

        # Deferred-norm pipeline: each projection group's norm+rope+PE
        # transposes are emitted AFTER the NEXT group's matmuls so the DVE
        # chain hides behind tensor work.
        pending = []

        def flush_pending():
            while pending:
                pending.pop(0)()

        def cg_group(g):
            """Context projection: kv chunks 2g, 2g+1."""
            sl = load_ckv(g * 256)
            cskp = evac_pool.tile([128, 2, 2 * D], BF16, tag="cskp")
            nc.gpsimd.dma_start(out=cskp, in_=csk[:, 2 * g:2 * g + 2, :])
            pk = p1.tile([128, 2, 2 * D], F32, tag="acc", name=f"cgk{g}")
            for k in range(NHD):
                for c in range(2):
                    nc.tensor.matmul(pk[:, c, :], lhsT=sl(k, c),
                                     rhs=wkv_sb[:, k, :],
                                     start=(k == 0 and c == 0),
                                     stop=(k == NHD - 1))
            ke = evac_pool.tile([128, 2, 2 * D], BF16, tag="ke")
            nc.scalar.copy(out=ke[:], in_=pk)
            flush_pending()
            pending.append(lambda ke=ke, r0=2 * g, cskp=cskp: knorm_pair(ke, r0, cskp))

        def qg_group(g):
            """Query-row projection: kv chunks 32+2g, 33+2g (q chunks 2g,
            2g+1): shared stationary feeds both wkv and wq."""
            sl = load_ckv(CTX + g * 256)
            cskp = evac_pool.tile([128, 2, 2 * D], BF16, tag="cskp")
            nc.gpsimd.dma_start(out=cskp, in_=csk[:, NCTX + 2 * g:NCTX + 2 * g + 2, :])
            csqp = evac_pool.tile([128, 2, 2 * D], BF16, tag="csqp")
            nc.gpsimd.dma_start(out=csqp, in_=csq[:, 2 * g:2 * g + 2, :])
            pk = p1.tile([128, 2, 2 * D], F32, tag="acc", name=f"qgk{g}")
            pq = [p1.tile([128, HPC * D], F32, tag="acc", name=f"qgq{g}_{c}")
                  for c in range(2)]
            for k in range(NHD):
                for c in range(2):
                    st = sl(k, c)
                    nc.tensor.matmul(pk[:, c, :], lhsT=st, rhs=wkv_sb[:, k, :],
                                     start=(k == 0 and c == 0),
                                     stop=(k == NHD - 1))
                    nc.tensor.matmul(pq[c], lhsT=st, rhs=wq_sb[:, k, :],
                                     start=(k == 0), stop=(k == NHD - 1))
            ke = evac_pool.tile([128, 2, 2 * D], BF16, tag="ke")
            nc.scalar.copy(out=ke[:], in_=pk)
            qes = []
            for c in range(2):
                qe = evac_pool.tile([128, HPC, D], BF16, tag="qe")
                nc.scalar.copy(out=qe[:], in_=pq[c])
                qes.append(qe)
            flush_pending()

            def norm(ke=ke, qes=qes, csqp=csqp, cskp=cskp, g=g):
                knorm_pair(ke, NCTX + 2 * g, cskp)
                for c in range(2):
                    qnorm(qes[c], 2 * g + c, csqp, c)
            pending.append(norm)

        # ---- attention ----
        # finalize (den matmuls, reciprocal, normalize, AG payload write) of
        # the previous stream is deferred and emitted after the next block of
        # dense PE work has started.
        finalize_pending = []

        def flush_finalize():
            while finalize_pending:
                finalize_pending.pop(0)()

        def make_finalize(j, h, o_acc, saccs):
            def fin():
                den = p1.tile([128, 512], F32, tag="acc", name=f"den{j}_{h}")
                for s in range(4):
                    nc.tensor.matmul(den, lhsT=ones_bf,
                                     rhs=saccs[:, s * 512:(s + 1) * 512],
                                     start=(s == 0), stop=(s == 3))
                pr = work_pool.tile([128, 512], F32, tag="dps")
                nc.vector.reciprocal_approx_fast(out=pr, in_=den)
                s3 = stg_pool.tile([128, 512], BF16, tag="stg")
                nc.vector.tensor_mul(s3, o_acc, pr)
                s3v = s3[:, :].rearrange("p (c q) -> p c q", c=4)
                if j == 0:
                    nc.gpsimd.dma_start(out=ag0_ins[h][:, :, :], in_=s3v)
                    nc.gpsimd.collective_compute(
                        "AllGather", ALU.bypass, ins=[ag0_ins[h][:, :, :]],
                        outs=[ag0_outs[h][:, :, :]],
                        replica_groups=[list(range(NCORES))])
                else:
                    nc.gpsimd.dma_start(out=ag_ins[j - 1][:, :, h, :], in_=s3v)
                    if h == HPC - 1:
                        nc.gpsimd.collective_compute(
                            "AllGather", ALU.bypass,
                            ins=[ag_ins[j - 1][:, :, :, :]],
                            outs=[ag_outs[j - 1][:, :, :, :]],
                            replica_groups=[list(range(NCORES))])
            return fin

        def attn_stream(j, h, split_at=None):
            """One (j,h) attention stream, software-pipelined one quad deep:
            the AV matmuls of quad q are emitted after the score matmuls of
            quad q+1 so the exp latency hides under PE work.
            split_at: optional chunk index at which to flush the projection
            norm pipeline (used once, to hide the last qg group's norm)."""
            r_list = list(range(NCTX + 4 + 4 * j))
            nq = len(r_list) // 4
            qTj = qT_sb[:, h, j * 512:(j + 1) * 512]
            o_acc = p1.tile([128, 512], F32, tag="acc", name=f"o{j}_{h}")
            saccs = saccs_pool.tile([128, 4 * 512], BF16, tag="sacc")

            def front(qd):
                """Score matmuls (+ causal-mask PSUM pre-bias via identity
                matmul) and exp for quad qd. Returns the exp'd ptw tile.
                The last (diagonal) quad computes only the unmasked column
                range [128*i:512] of chunk i: the fully-masked prefix is
                skipped in scores, exp, AV and softmax-sum alike."""
                ptw = pt_pool.tile([128, 4 * 512], BF16, tag="ptw")
                diag = qd == nq - 1
                sts = []
                for pi in range(2):
                    rA = r_list[qd * 4 + 2 * pi]
                    rB = r_list[qd * 4 + 2 * pi + 1]
                    st = p2.tile([128, 1024], F32, tag="st")
                    for idx, r in ((0, rA), (1, rB)):
                        if diag:
                            lo = 128 * (2 * pi + idx)
                            nc.tensor.matmul(
                                st[:, idx * 512 + lo:idx * 512 + lo + 128],
                                lhsT=identb, rhs=msk_sb[:, 384:512],
                                start=True, stop=False)
                            nc.tensor.matmul(
                                st[:, idx * 512 + lo:(idx + 1) * 512],
                                lhsT=kT_sb[:, r * 128:(r + 1) * 128],
                                rhs=qTj[:, lo:512], start=False, stop=True)
                        else:
                            nc.tensor.matmul(
                                st[:, idx * 512:(idx + 1) * 512],
                                lhsT=kT_sb[:, r * 128:(r + 1) * 128],
                                rhs=qTj, start=True, stop=True)
                    sts.append(st)
                if diag:
                    for slot in range(4):
                        lo = 128 * slot
                        nc.scalar.activation(
                            out=ptw[:, slot * 512 + lo:(slot + 1) * 512],
                            in_=sts[slot // 2][:, (slot % 2) * 512 + lo:
                                               (slot % 2 + 1) * 512],
                            func=mybir.ActivationFunctionType.Exp,
                            bias=zbias, scale=SCALE)
                    return ptw
                for pi in range(2):
                    # Every 4th score pair (never a masked/diagonal one) goes
                    # through a bf16-Schraudolph exp on DVE instead of ACT:
                    # bf16_bits(exp(x*SCALE)) ~= int16(128/ln2*SCALE*x + B).
                    # The constant bias cancels in the softmax normalization;
                    # the ~1.8% rms sawtooth on 1/4 of the weights is well
                    # inside the error budget, and the ACT engine stops
                    # pacing the attention phase.
                    if pi == 0 and qd % 2 == 0:
                        nc.vector.tensor_scalar(
                            out=ptw[:, 0:1024].bitcast(mybir.dt.int16),
                            in0=sts[pi][:], scalar1=SCHR_A, scalar2=SCHR_B,
                            op0=ALU.mult, op1=ALU.add)
                    else:
                        nc.scalar.activation(
                            out=ptw[:, pi * 1024:(pi + 1) * 1024],
                            in_=sts[pi][:],
                            func=mybir.ActivationFunctionType.Exp,
                            bias=zbias, scale=SCALE)
                return ptw

            def back(qd, ptw):
                diag = qd == nq - 1
                for slot in range(4):
                    r = r_list[qd * 4 + slot]
                    lo = 128 * slot if diag else 0
                    nc.tensor.matmul(
                        o_acc[:, lo:512], lhsT=v_sb[:, r, :],
                        rhs=ptw[:, slot * 512 + lo:(slot + 1) * 512],
                        start=(qd == 0 and slot == 0),
                        stop=(qd == nq - 1 and slot == 3))
                if qd == 0:
                    nc.vector.tensor_copy(out=saccs, in_=ptw)
                elif diag:
                    for slot in range(4):
                        lo = 128 * slot
                        nc.vector.tensor_add(
                            saccs[:, slot * 512 + lo:(slot + 1) * 512],
                            saccs[:, slot * 512 + lo:(slot + 1) * 512],
                            ptw[:, slot * 512 + lo:(slot + 1) * 512])
                else:
                    nc.vector.tensor_add(saccs, saccs, ptw)

            prev = None
            for qd in range(nq):
                if split_at is not None and qd * 4 == split_at:
                    flush_pending()
                ptw = front(qd)
                if qd == 1:
                    flush_finalize()
                if prev is not None:
                    back(qd - 1, prev)
                prev = ptw
            back(nq - 1, prev)
            finalize_pending.append(make_finalize(j, h, o_acc, saccs))

        def at_load(jq, ats, hv):
            qc2, lo = hv // 2, (hv % 2) * 4
            at = at_pool.tile([128, 4, HPC, 128], BF16, tag="at")
            nc.sync.dma_start(
                out=at,
                in_=ag_outs[jq - 1][lo * 128:(lo + 4) * 128, qc2, :, :]
                .rearrange("(c p) h q -> p c h q", p=128))
            ats[hv] = at

        def oproj_prefetch(jq):
            """Issue the first two attn^T tile loads for oproj_block(jq)
            early, so its first matmuls don't expose the DMA latency."""
            ats = {}
            at_load(jq, ats, 0)
            at_load(jq, ats, 1)
            return ats

        def oproj_block(jq, ats):
            """o_proj for q tile jq (jq >= 1) from the AllGathered attn^T.
            Tiles load per (qc, 4-core half) as 1KB-contiguous segments."""
            for qc2 in range(4):
                qc = 4 * jq + qc2
                po = p1.tile([128, HPC * D], F32, tag="acc", name=f"po{qc}")
                for half in range(2):
                    at = ats.pop(qc2 * 2 + half)
                    for c4 in range(4):
                        for h2 in range(HPC):
                            gg = (half * 4 + c4) * HPC + h2
                            nc.tensor.matmul(po, lhsT=at[:, c4, h2, :],
                                             rhs=wo_sb[:, gg, :],
                                             start=(gg == 0), stop=(gg == H - 1))
                    if qc2 == 0 and half == 0:
                        flush_finalize()
                    if qc2 * 2 + half + 2 < 8:
                        at_load(jq, ats, qc2 * 2 + half + 2)
                ot = stg_pool.tile([128, HPC * D], F32, tag="ot")
                nc.scalar.copy(out=ot, in_=po)
                nc.sync.dma_start(out=out_ext[qc * 128:(qc + 1) * 128, :], in_=ot)

        def oproj_block0():
            """o_proj for q tile 0, head-major: the h2<3 contraction chunks
            run while AG(0, h=3) is still in flight, so only the last-quarter
            matmuls trail the final collective."""
            pos = [p1.tile([128, HPC * D], F32, tag="acc", name=f"po0_{qc2}")
                   for qc2 in range(4)]
            ats = {}

            def at0_load(h2, qc2):
                at = at_pool.tile([128, NCORES, 128], BF16, tag="at")
                nc.sync.dma_start(
                    out=at,
                    in_=ag0_outs[h2][:, qc2, :].rearrange("(c p) q -> p c q",
                                                          p=128))
                ats[(h2, qc2)] = at

            at0_load(0, 0)
            at0_load(0, 1)
            order = [(h2, qc2) for h2 in range(HPC) for qc2 in range(4)]
            for oi, (h2, qc2) in enumerate(order):
                at = ats.pop((h2, qc2))
                po = pos[qc2]
                for cc in range(NCORES):
                    nc.tensor.matmul(po, lhsT=at[:, cc, :],
                                     rhs=wo_sb[:, cc * HPC + h2, :],
                                     start=(h2 == 0 and cc == 0),
                                     stop=(h2 == HPC - 1 and cc == NCORES - 1))
                if oi + 2 < len(order):
                    at0_load(*order[oi + 2])
                if h2 == HPC - 1:
                    ot = stg_pool.tile([128, HPC * D], F32, tag="ot")
                    nc.scalar.copy(out=ot, in_=po)
                    nc.sync.dma_start(
                        out=out_ext[qc2 * 128:(qc2 + 1) * 128, :], in_=ot)

        # ---- main sequence ----
        cg_group(0)
        nc.sync.dma_start(out=msk_sb[:], in_=mskd[:, :])
        for g in range(1, 10):
            cg_group(g)
        nc.scalar.dma_start(out=wq_sb[:], in_=wq3[:, :, :])
        for g in range(10, 16):
            cg_group(g)
        # qg group 0 (q chunks 0,1 / kv 32,33) runs LAST so its deferred norm
        # can flush inside the first attention stream (which needs q chunks
        # 12-15 and all kv, but touches kv>=32 only from its 9th quad on).
        qg_group(1)
        qg_group(2)
        nc.sync.dma_start(out=wo_sb[:], in_=wo3[:, :, :])
        for g in (3, 4, 5, 6, 7, 0):
            qg_group(g)

        attn_stream(3, 0, split_at=32)
        attn_stream(3, 1)
        attn_stream(3, 2)
        attn_stream(3, 3)
        attn_stream(2, 0)
        attn_stream(2, 1)
        pre = oproj_prefetch(3)
        attn_stream(2, 2)
        attn_stream(2, 3)
        oproj_block(3, pre)
        attn_stream(1, 0)
        attn_stream(1, 1)
        pre = oproj_prefetch(2)
        attn_stream(1, 2)
        attn_stream(1, 3)
        oproj_block(2, pre)
        attn_stream(0, 0)
        attn_stream(0, 1)
        pre = oproj_prefetch(1)
        attn_stream(0, 2)
        attn_stream(0, 3)
        flush_finalize()  # fires AG(0,3) immediately; oproj(1) hides it
        oproj_block(1, pre)
        oproj_block0()

    nc.compile()
    return nc


def _tile_hid(a):
    """[HID, C] -> [128, NHD, C] with (p, k, c) = a[k*128+p, c]."""
    return np.ascontiguousarray(
        a.reshape(NHD, 128, a.shape[1]).transpose(1, 0, 2)).astype(bfloat16)


def _host_prep(context, query, w_qkv, w_o, q_norm_w, k_norm_w):
    context = np.asarray(context, dtype=np.float32)
    query = np.asarray(query, dtype=np.float32)
    w_qkv = np.asarray(w_qkv, dtype=np.float32)
    w_o = np.asarray(w_o, dtype=np.float32)
    q_norm_w = np.asarray(q_norm_w, dtype=np.float32)
    k_norm_w = np.asarray(k_norm_w, dtype=np.float32)

    ck = np.concatenate([context, query], axis=0)  # [KV, HID]
    ckT = np.ascontiguousarray(ck.T)  # [HID, KV] f32
    # [g][p][k][c] blocks of 256 kv columns (one projection group each)
    ckT4 = np.ascontiguousarray(
        ckT.reshape(NHD, 128, KV // 256, 256).transpose(2, 1, 0, 3)
    ).astype(bfloat16)

    wq = w_qkv[:, :H * D]
    wk = w_qkv[:, H * D:H * D + KVH * D]
    wv = w_qkv[:, H * D + KVH * D:]

    half = D // 2
    inv_freq = (1.0 / (THETA ** (np.arange(0, half, dtype=np.float32) / half))
                ).astype(np.float32)
    pos = np.arange(KV, dtype=np.float32)
    freqs = pos[:, None] * inv_freq[None, :]
    cosf, sinf = np.cos(freqs), np.sin(freqs)  # [KV, 64]

    def cs4(w):
        # [cos*w1 | sin*w2 | cos*w2 | sin*w1], w1 = w[:64], w2 = w[64:]
        return np.concatenate([cosf * w[None, :half], sinf * w[None, half:],
                               cosf * w[None, half:], sinf * w[None, :half]],
                              axis=1).astype(np.float32)  # [KV, 2D]

    cs4q = cs4(q_norm_w)[CTX:]  # [QL, 2D]
    cs4k = cs4(k_norm_w)  # [KV, 2D]
    csq_t = np.ascontiguousarray(
        cs4q.reshape(NQC, 128, 2 * D).transpose(1, 0, 2)).astype(bfloat16)
    csk_t = np.ascontiguousarray(
        cs4k.reshape(NKV, 128, 2 * D).transpose(1, 0, 2)).astype(bfloat16)

    p = np.arange(128)[:, None]
    t = np.arange(896)[None, :]
    msk = np.where(p <= t - 384, 0.0, MASKVAL).astype(bfloat16)  # [128, 896]

    in_maps = []
    for c in range(NCORES):
        in_maps.append({
            "ckT4": ckT4,
            "wq3": _tile_hid(wq[:, c * HPC * D:(c + 1) * HPC * D]),
            "wkv3": _tile_hid(np.concatenate(
                [wk[:, c * D:(c + 1) * D], wv[:, c * D:(c + 1) * D]], axis=1)),
            "wo3": _tile_hid(w_o[:, c * HPC * D:(c + 1) * HPC * D]),
            "csq": csq_t,
            "csk": csk_t,
            "msk": msk,
        })
    return in_maps


def kernel(context, query, w_qkv, w_o, q_norm_w, k_norm_w, **kw):
    if "nc" not in _STATE:
        _STATE["nc"] = _build()
    nc = _STATE["nc"]
    in_maps = _host_prep(context, query, w_qkv, w_o, q_norm_w, k_norm_w)
    res = run_bass_kernel_spmd(nc, in_maps, list(range(NCORES)), **kw)
    out = np.concatenate([np.asarray(res.results[c]["out"]) for c in range(NCORES)],
                         axis=1)
    if kw:
        return out.astype(np.float32), res
    return out.astype(np.float32)
